# revision 10
# baseline (speedup 1.0000x reference)
"""Trainium2 Bass kernel for ALBERT attention (B=2, S=2048, H=1024, NH=16).

Sharding over 8 NeuronCores: 2 batches x 4 head-groups (tensor parallel over
heads within each batch).  Core c handles batch b = c//4 and heads
[4g, 4g+4) where g = c%4.  After the output projection, a ReduceScatter(add)
over each batch's 4-core group sums the per-headgroup partial projections and
hands core (b, g) the token chunk [512g, 512g+512), on which it applies the
residual + bias + LayerNorm and writes its [512, 1024] output slice.

Device-side dataflow (per core), all fp32 with float32r matmuls:
  hsT [1024, 2048]  (host-pretransposed hidden states of its batch)
  qT/kT = W.T-slices @ hsT          (feature-major, head pairs stacked 64+64)
  v     = hsT.T @ WvT-slice         (token-major) + ones column per head
  per head: scoresT[key, q] = k @ qT  (two heads run row-packed, K=64)
            expT = exp(0.125*scoresT + mask[key])     (ScalarE, mask as bias)
            ctx~T[65, q] = [v | 1].T @ expT           (PSUM-accumulated over key)
            ctxT = ctx~T[:64] / ctx~T[64]             (recip + K=1 bcast matmul)
  partial = ctxT.T @ WdT-slice      -> DRAM -> ReduceScatter(groups of 4)
  out = LN(rs + hs_own + bd) * gamma + beta
"""

import os
import sys

import numpy as np

for _p in ("/opt/trn_rl_repo",):
    if _p not in sys.path:
        sys.path.insert(0, _p)

import concourse.bass as bass
import concourse.mybir as mybir
import concourse.tile as tile
from concourse import bacc
from concourse.bass import ts

F32 = mybir.dt.float32
F32R = mybir.dt.float32r

H, NH, HD = 1024, 16, 64
B, S = 2, 2048
NCORES = 8
GROUPS = [[0, 1, 2, 3], [4, 5, 6, 7]]
CHUNK = 512          # output tokens per core
HEADS_PER_CORE = 4   # 2 pairs
EPS = 1e-12


def r(ap):
    """Matmul-input tiles are natively float32r; passthrough."""
    return ap


def build_nc():
    nc = bacc.Bacc(
        "TRN2",
        target_bir_lowering=False,
        debug=False,
        num_devices=NCORES,
    )

    hsT_d = nc.dram_tensor("hsT", [H, S], F32R, kind="ExternalInput")
    hso_d = nc.dram_tensor("hs_own", [CHUNK, H], F32, kind="ExternalInput")
    wq_d = nc.dram_tensor("wqT", [H, 256], F32R, kind="ExternalInput")
    wk_d = nc.dram_tensor("wkT", [H, 256], F32R, kind="ExternalInput")
    wv_d = nc.dram_tensor("wvT", [H, 256], F32R, kind="ExternalInput")
    wd_d = nc.dram_tensor("wdT", [256, H], F32R, kind="ExternalInput")
    bq_d = nc.dram_tensor("bq_s", [256], F32, kind="ExternalInput")
    bk_d = nc.dram_tensor("bk_s", [256], F32, kind="ExternalInput")
    bv_d = nc.dram_tensor("bv_s", [256], F32R, kind="ExternalInput")
    bd_d = nc.dram_tensor("bd_f", [H], F32R, kind="ExternalInput")
    gam_d = nc.dram_tensor("gamma_f", [H], F32R, kind="ExternalInput")
    bet_d = nc.dram_tensor("beta_f", [H], F32R, kind="ExternalInput")
    msk_d = nc.dram_tensor("mask_b", [S], F32, kind="ExternalInput")
    ones_d = nc.dram_tensor("ones_f", [128, 128], F32R, kind="ExternalInput")
    out_d = nc.dram_tensor("out_chunk", [CHUNK, H], F32, kind="ExternalOutput")

    with tile.TileContext(nc) as tc:
        _body(tc, hsT_d, hso_d, wq_d, wk_d, wv_d, wd_d, bq_d, bk_d, bv_d,
              bd_d, gam_d, bet_d, msk_d, ones_d, out_d)
    nc.compile()
    return nc


def _body(tc, hsT_d, hso_d, wq_d, wk_d, wv_d, wd_d, bq_d, bk_d, bv_d,
          bd_d, gam_d, bet_d, msk_d, ones_d, out_d):
    nc = tc.nc
    Exp = mybir.ActivationFunctionType.Exp
    Sqrt = mybir.ActivationFunctionType.Sqrt
    add_op = mybir.AluOpType.add
    sub_op = mybir.AluOpType.subtract
    mul_op = mybir.AluOpType.mult

    # ---------------- persistent SBUF ----------------
    persist = tc.alloc_tile_pool(name="persist", bufs=1)
    qT_sb = persist.tile([128, 2, S], F32R)       # [dim-in-pair, pair, tok]
    kT_sb = persist.tile([128, 2, S], F32R)
    vaug_sb = persist.tile([128, 16, 4, 65], F32R)  # [key-in-tile, keytile, head, 64v+1]
    ctxT_sb = persist.tile([128, 2, S], F32R)     # normalized ctx, feature-major
    ctxodd_sb = persist.tile([64, 2, S], F32R)    # odd heads before partition shift
    wd_sb = persist.tile([128, 2, H], F32R)
    bq_sb = persist.tile([128, 2], F32)
    bk_sb = persist.tile([128, 2], F32)
    msk_sb = persist.tile([128, 16], F32)
    ones_sb = persist.tile([128, 128], F32R)
    bv_sb = persist.tile([1, 256], F32R)
    bd_sb = persist.tile([1, H], F32R)
    gam_sb = persist.tile([1, H], F32R)
    bet_sb = persist.tile([1, H], F32R)
    eps_sb = persist.tile([128, 1], F32)

    nc.vector.memset(eps_sb, EPS)
    nc.sync.dma_start(out=ones_sb, in_=ones_d.ap())
    # fill the per-head ones column of v~ from the DRAM ones tensor
    nc.sync.dma_start(
        out=vaug_sb[:, :, :, 64:65],
        in_=ones_d.ap()[:, 0:64].rearrange("p (a b) -> p a b", a=16).unsqueeze(3),
    )

    nc.sync.dma_start(out=bq_sb, in_=bq_d.ap().rearrange("(c p) -> p c", p=128))
    nc.sync.dma_start(out=bk_sb, in_=bk_d.ap().rearrange("(c p) -> p c", p=128))
    nc.sync.dma_start(out=bv_sb, in_=bv_d.ap().unsqueeze(0))
    nc.sync.dma_start(out=bd_sb, in_=bd_d.ap().unsqueeze(0))
    nc.sync.dma_start(out=gam_sb, in_=gam_d.ap().unsqueeze(0))
    nc.sync.dma_start(out=bet_sb, in_=bet_d.ap().unsqueeze(0))
    nc.sync.dma_start(out=msk_sb, in_=msk_d.ap().rearrange("(t p) -> p t", p=128))

    # ---------------- load pool (released after QKV) ----------------
    load = tc.alloc_tile_pool(name="load", bufs=1)
    hsT_sb = load.tile([128, 8, S], F32R)         # [feat-in-chunk, featchunk, tok]
    wq_sb = load.tile([128, 8, 256], F32R)
    wk_sb = load.tile([128, 8, 256], F32R)
    wv_sb = load.tile([128, 8, 256], F32R)

    nc.sync.dma_start(out=wq_sb, in_=wq_d.ap().rearrange("(c p) d -> p c d", p=128))
    nc.sync.dma_start(out=wk_sb, in_=wk_d.ap().rearrange("(c p) d -> p c d", p=128))
    nc.sync.dma_start(out=wv_sb, in_=wv_d.ap().rearrange("(c p) d -> p c d", p=128))
    nc.sync.dma_start(out=wd_sb, in_=wd_d.ap().rearrange("(c p) d -> p c d", p=128))
    # hidden states arrive in token-chunk slices so compute can start early
    hsT_src = hsT_d.ap().rearrange("(c p) (q w) -> q p c w", p=128, w=512)
    for t4 in range(4):
        nc.sync.dma_start(out=hsT_sb[:, :, ts(t4, 512)], in_=hsT_src[t4])

    # ---------------- QKV projections ----------------
    qkv_ps = tc.alloc_tile_pool(name="qkv_ps", bufs=3, space="PSUM")

    for t4 in range(4):
        for pr in range(2):
            for which, w_sb, b_sb, o_sb in (
                (0, wq_sb, bq_sb, qT_sb),
                (1, wk_sb, bk_sb, kT_sb),
            ):
                ps = qkv_ps.tile([128, 512], F32, tag="qk_ps")
                for kc in range(8):
                    nc.tensor.matmul(
                        ps,
                        lhsT=r(w_sb[:, kc, ts(pr, 128)]),
                        rhs=r(hsT_sb[:, kc, ts(t4, 512)]),
                        start=(kc == 0),
                        stop=(kc == 7),
                    )
                nc.vector.tensor_scalar_add(
                    out=o_sb[:, pr, ts(t4, 512)], in0=ps,
                    scalar1=b_sb[:, pr:pr + 1],
                )
        for tt in range(4):          # token tiles of 128 within this 512 chunk
            t16 = 4 * t4 + tt
            ps = qkv_ps.tile([128, 256], F32, tag="v_ps")
            for kc in range(8):
                nc.tensor.matmul(
                    ps,
                    lhsT=r(hsT_sb[:, kc, ts(t16, 128)]),
                    rhs=r(wv_sb[:, kc, :]),
                    start=(kc == 0),
                    stop=False,
                )
            nc.tensor.matmul(        # + bv broadcast over tokens (K=1)
                ps,
                lhsT=r(ones_sb[0:1, 0:128]),
                rhs=r(bv_sb[0:1, :]),
                start=False,
                stop=True,
            )
            nc.vector.tensor_copy(
                out=vaug_sb[:, t16, :, 0:64],
                in_=ps.rearrange("p (h d) -> p h d", h=4),
            )

    qkv_ps.release()
    load.release()

    # ---------------- attention ----------------
    at_sc = tc.alloc_tile_pool(name="at_sc", bufs=2, space="PSUM")    # 4 banks
    at_ctx = tc.alloc_tile_pool(name="at_ctx", bufs=1, space="PSUM")  # 2 banks
    at_rbc = tc.alloc_tile_pool(name="at_rbc", bufs=2, space="PSUM")  # 2 banks
    at_sb = tc.alloc_tile_pool(name="at_sb", bufs=3)

    for pr in range(2):
        for qc in range(4):
            ctx_e = at_ctx.tile([65, 512], F32, tag="ctx_e")
            ctx_o = at_ctx.tile([65, 512], F32, tag="ctx_o")
            for kt in range(16):
                sc = at_sc.tile([128, 1024], F32, tag="sc")
                nc.tensor.matmul(
                    sc[:, 0:512],
                    lhsT=r(kT_sb[0:64, pr, ts(kt, 128)]),
                    rhs=r(qT_sb[0:64, pr, ts(qc, 512)]),
                )
                nc.tensor.matmul(
                    sc[:, 512:1024],
                    lhsT=r(kT_sb[64:128, pr, ts(kt, 128)]),
                    rhs=r(qT_sb[64:128, pr, ts(qc, 512)]),
                )
                ex = at_sb.tile([128, 1024], F32R, tag="ex")
                nc.scalar.activation(
                    out=ex, in_=sc[:, :], func=Exp,
                    bias=msk_sb[:, kt:kt + 1], scale=0.125,
                )
                nc.tensor.matmul(
                    ctx_e,
                    lhsT=r(vaug_sb[:, kt, 2 * pr, :]),
                    rhs=r(ex[:, 0:512]),
                    start=(kt == 0), stop=(kt == 15),
                )
                nc.tensor.matmul(
                    ctx_o,
                    lhsT=r(vaug_sb[:, kt, 2 * pr + 1, :]),
                    rhs=r(ex[:, 512:1024]),
                    start=(kt == 0), stop=(kt == 15),
                )
            # normalize: ctxT_h = ctx~[:64] / ctx~[64]
            for hodd, ctx_ps in ((0, ctx_e), (1, ctx_o)):
                lrec = at_sb.tile([128, 512], F32, tag="lrec")
                nc.vector.tensor_copy(out=lrec[64:65, :], in_=ctx_ps[64:65, :])
                nc.vector.reciprocal(out=lrec[64:65, :], in_=lrec[64:65, :])
                lrecr = at_sb.tile([128, 512], F32R, tag="lrecr")
                nc.vector.tensor_copy(out=lrecr[64:65, :], in_=lrec[64:65, :])
                rbc = at_rbc.tile([64, 512], F32, tag="rbc")
                nc.tensor.matmul(
                    rbc,
                    lhsT=r(ones_sb[64:65, 0:64]),
                    rhs=r(lrecr[64:65, :]),
                    tile_position=(64, 0),
                )
                rbs = at_sb.tile([64, 512], F32, tag="rbs")
                nc.vector.tensor_copy(out=rbs, in_=rbc)
                if hodd == 0:
                    dst = ctxT_sb[0:64, pr, ts(qc, 512)]
                else:
                    dst = ctxodd_sb[0:64, pr, ts(qc, 512)]
                nc.vector.tensor_tensor(
                    out=dst, in0=ctx_ps[0:64, :], in1=rbs, op=mul_op,
                )
        # partition-shift odd head rows into ctxT rows 64..127 (DMA can cross
        # partitions; compute engines cannot)
        nc.sync.dma_start(out=ctxT_sb[64:128, pr, :], in_=ctxodd_sb[0:64, pr, :])

    at_sb.release()
    at_rbc.release()
    at_ctx.release()
    at_sc.release()

    # ---------------- dense projection -> DRAM partial ----------------
    dram = tc.alloc_tile_pool(name="dram", bufs=1, space="DRAM")
    partial_d = dram.tile([S, H], F32)
    rs_out = dram.tile([CHUNK, H], F32)

    d_ps = tc.alloc_tile_pool(name="d_ps", bufs=3, space="PSUM")
    d_sb = tc.alloc_tile_pool(name="d_sb", bufs=3)
    for t16 in range(16):
        pj = d_sb.tile([128, H], F32, tag="pj")
        for nh in range(2):
            ps = d_ps.tile([128, 512], F32, tag="d_ps")
            for pr in range(2):
                nc.tensor.matmul(
                    ps,
                    lhsT=r(ctxT_sb[:, pr, ts(t16, 128)]),
                    rhs=r(wd_sb[:, pr, ts(nh, 512)]),
                    start=(pr == 0),
                    stop=(pr == 1),
                )
            nc.vector.tensor_copy(out=pj[:, ts(nh, 512)], in_=ps)
        nc.sync.dma_start(
            out=partial_d[ts(t16, 128), :], in_=pj,
        )

    # ---------------- cross-core reduce ----------------
    nc.gpsimd.collective_compute(
        "ReduceScatter",
        add_op,
        replica_groups=GROUPS,
        ins=[partial_d[:].flatten()],
        outs=[rs_out[:].flatten()],
    )

    # ---------------- residual + bias + LayerNorm ----------------
    fin_sb = tc.alloc_tile_pool(name="fin_sb", bufs=2)
    rs_sb = fin_sb.tile([128, 4, H], F32, bufs=1)
    hso_sb = fin_sb.tile([128, 4, H], F32, bufs=1)
    nc.sync.dma_start(out=rs_sb, in_=rs_out[:].rearrange("(t p) d -> p t d", p=128))
    nc.sync.dma_start(out=hso_sb, in_=hso_d.ap().rearrange("(t p) d -> p t d", p=128))

    # broadcast bd / gamma / beta across partitions via K=1 matmuls
    bcast_ps = d_ps  # reuse pool
    bdb_sb = fin_sb.tile([128, H], F32, bufs=1)
    gmb_sb = fin_sb.tile([128, H], F32, bufs=1)
    btb_sb = fin_sb.tile([128, H], F32, bufs=1)
    for src, dst in ((bd_sb, bdb_sb), (gam_sb, gmb_sb), (bet_sb, btb_sb)):
        pb = bcast_ps.tile([128, H], F32, tag="bc_ps", bufs=1)
        for nh in range(2):
            nc.tensor.matmul(
                pb[:, ts(nh, 512)],
                lhsT=r(ones_sb[0:1, 0:128]),
                rhs=r(src[0:1, ts(nh, 512)]),
            )
        nc.vector.tensor_copy(out=dst, in_=pb)

    out_src = out_d.ap().rearrange("(t p) d -> t p d", p=128)
    for t4 in range(4):
        x = fin_sb.tile([128, H], F32, tag="x")
        nc.vector.tensor_tensor(out=x, in0=rs_sb[:, t4, :], in1=hso_sb[:, t4, :], op=add_op)
        nc.vector.tensor_tensor(out=x, in0=x, in1=bdb_sb, op=add_op)
        stats = fin_sb.tile([128, 2, 6], F32, tag="stats")
        for i in range(2):
            nc.vector.bn_stats(out=stats[:, i, :], in_=x[:, ts(i, 512)])
        mv = fin_sb.tile([128, 2], F32, tag="mv")
        nc.vector.bn_aggr(out=mv, in_=stats)
        sd = fin_sb.tile([128, 1], F32, tag="sd")
        nc.scalar.activation(out=sd, in_=mv[:, 1:2], func=Sqrt, bias=eps_sb, scale=1.0)
        rinv = fin_sb.tile([128, 1], F32, tag="rinv")
        nc.vector.reciprocal(out=rinv, in_=sd)
        nc.vector.tensor_scalar(
            out=x, in0=x, scalar1=mv[:, 0:1], scalar2=rinv,
            op0=sub_op, op1=mul_op,
        )
        nc.vector.tensor_tensor(out=x, in0=x, in1=gmb_sb, op=mul_op)
        nc.vector.tensor_tensor(out=x, in0=x, in1=btb_sb, op=add_op)
        nc.sync.dma_start(out=out_src[t4], in_=x)

    fin_sb.release()
    d_sb.release()
    d_ps.release()
    dram.release()
    persist.release()


_NC_CACHE = {}


def _get_nc():
    if "nc" not in _NC_CACHE:
        _NC_CACHE["nc"] = build_nc()
    return _NC_CACHE["nc"]


def shard_inputs(inputs):
    hs = np.ascontiguousarray(np.asarray(inputs["hidden_states"], dtype=np.float32))
    mask = np.asarray(inputs["attention_mask"], dtype=np.float32)
    Wq = np.asarray(inputs["Wq"], dtype=np.float32)
    Wk = np.asarray(inputs["Wk"], dtype=np.float32)
    Wv = np.asarray(inputs["Wv"], dtype=np.float32)
    Wd = np.asarray(inputs["Wd"], dtype=np.float32)
    bq = np.asarray(inputs["bq"], dtype=np.float32)
    bk = np.asarray(inputs["bk"], dtype=np.float32)
    bv = np.asarray(inputs["bv"], dtype=np.float32)
    bd = np.ascontiguousarray(np.asarray(inputs["bd"], dtype=np.float32))
    gam = np.ascontiguousarray(np.asarray(inputs["ln_gamma"], dtype=np.float32))
    bet = np.ascontiguousarray(np.asarray(inputs["ln_beta"], dtype=np.float32))

    hsT = [np.ascontiguousarray(hs[b].T) for b in range(B)]
    mask_b = [np.ascontiguousarray(mask[b, 0, 0, :]) for b in range(B)]

    in_maps = []
    for c in range(NCORES):
        b, g = c // 4, c % 4
        sl = slice(256 * g, 256 * g + 256)
        in_maps.append({
            "hsT": hsT[b],
            "hs_own": np.ascontiguousarray(hs[b, CHUNK * g: CHUNK * (g + 1)]),
            "wqT": np.ascontiguousarray(Wq[sl, :].T),
            "wkT": np.ascontiguousarray(Wk[sl, :].T),
            "wvT": np.ascontiguousarray(Wv[sl, :].T),
            "wdT": np.ascontiguousarray(Wd[:, sl].T),
            "bq_s": np.ascontiguousarray(bq[sl]),
            "bk_s": np.ascontiguousarray(bk[sl]),
            "bv_s": np.ascontiguousarray(bv[sl]),
            "bd_f": bd,
            "gamma_f": gam,
            "beta_f": bet,
            "mask_b": mask_b[b],
            "ones_f": _ONES,
        })
    return in_maps


def assemble(results):
    out = np.zeros((B, S, H), np.float32)
    for c in range(NCORES):
        b, g = c // 4, c % 4
        out[b, CHUNK * g: CHUNK * (g + 1), :] = results[c]["out_chunk"]
    return out


_ONES = np.ones((128, 128), np.float32)

LAST_RESULT = None


def kernel(**inputs):
    global LAST_RESULT
    from concourse.bass_utils import run_bass_kernel_spmd

    nc = _get_nc()
    in_maps = shard_inputs(inputs)
    trace = bool(int(os.environ.get("KERNEL_TRACE", "0")))
    res = run_bass_kernel_spmd(nc, in_maps, list(range(NCORES)), trace=trace)
    LAST_RESULT = res
    return assemble(res.results)


def simulate(inputs):
    """CoreSim-based check (no hardware)."""
    from concourse.bass_interp import MultiCoreSim

    nc = _get_nc()
    in_maps = shard_inputs(inputs)
    sim = MultiCoreSim(nc, NCORES)
    for c in range(NCORES):
        for k, v in in_maps[c].items():
            sim.cores[c].tensor(k)[:] = v
    sim.simulate(check_with_hw=False)
    results = [{"out_chunk": np.array(sim.cores[c].tensor("out_chunk"))}
               for c in range(NCORES)]
    return assemble(results)


# revision 14
# speedup vs baseline: 1.4220x; 1.4220x over previous
"""Trainium2 Bass kernel for ALBERT attention (B=2, S=2048, H=1024, NH=16).

Sharding over 8 NeuronCores: 2 batches x 4 head-groups (tensor parallel over
heads within each batch).  Core c handles batch b = c//4 and heads
[4g, 4g+4) where g = c%4.  The kernel pipelines over four 512-token chunks:
for each chunk it runs attention (both head pairs), the partial output
projection, and a ReduceScatter(add) over the batch's 4-core group that both
sums the head-group partials and scatters token ownership; the RS of chunk i
overlaps the attention of chunk i+1.  Core (b, g) ends up owning token rows
512*qc + 128*g .. +128 for qc in 0..3, applies residual + bias + LayerNorm,
and writes those four [128, 1024] slices.

Matmuls run in bf16 (inputs host-cast; fp32 PSUM accumulation).  The softmax
normalization (1/l) and the tiny K=1 broadcast matmuls stay float32r.

Per-core dataflow:
  hsT [1024, 2048] bf16   (host-pretransposed hidden states of its batch)
  qT/kT = W.T-slices @ hsT          (feature-major, head pairs stacked 64+64)
  v     = hsT.T @ WvT-slice         (token-major) + ones column per head
  per 512-token q chunk, per head pair, per 128-key tile:
      scoresT[key, q] = k @ qT   (two heads row-packed, K=64)
      expT = exp(0.125*scoresT + mask[key])   (ScalarE, mask as bias)
      ctx~T[65, q] += [v | 1].T @ expT        (PSUM-accumulated over keys)
  ctxT_h = ctx~T[:64] / ctx~T[64]   (merged reciprocal + K=1 bcast matmul)
  partial(qc) = ctxT(qc).T @ WdT-slice -> ReduceScatter(qc) over group of 4
  out = LN(rs + hs_own + bd) * gamma + beta
"""

import os
import sys

import numpy as np

for _p in ("/opt/trn_rl_repo",):
    if _p not in sys.path:
        sys.path.insert(0, _p)

import concourse.bass as bass
import concourse.mybir as mybir
import concourse.tile as tile
from concourse import bacc
from concourse.bass import ts

F32 = mybir.dt.float32
F32R = mybir.dt.float32r
BF16 = mybir.dt.bfloat16

H, NH, HD = 1024, 16, 64
B, S = 2, 2048
NCORES = 8
GROUPS = [[0, 1, 2, 3], [4, 5, 6, 7]]
CHUNK = 512          # tokens per pipelined chunk (and per-core output rows)
EPS = 1e-12


def build_nc():
    nc = bacc.Bacc(
        "TRN2",
        target_bir_lowering=False,
        debug=False,
        num_devices=NCORES,
    )

    hsT_d = nc.dram_tensor("hsT", [H, S], BF16, kind="ExternalInput")
    hso_d = nc.dram_tensor("hs_own", [4, 128, H], F32, kind="ExternalInput")
    wq_d = nc.dram_tensor("wqT", [H, 256], BF16, kind="ExternalInput")
    wk_d = nc.dram_tensor("wkT", [H, 256], BF16, kind="ExternalInput")
    wv_d = nc.dram_tensor("wvT", [H, 256], BF16, kind="ExternalInput")
    wd_d = nc.dram_tensor("wdT", [256, H], BF16, kind="ExternalInput")
    bq_d = nc.dram_tensor("bq_s", [256], F32, kind="ExternalInput")
    bk_d = nc.dram_tensor("bk_s", [256], F32, kind="ExternalInput")
    bv_d = nc.dram_tensor("bv_s", [256], F32R, kind="ExternalInput")
    bd_d = nc.dram_tensor("bd_f", [H], F32R, kind="ExternalInput")
    gam_d = nc.dram_tensor("gamma_f", [H], F32R, kind="ExternalInput")
    bet_d = nc.dram_tensor("beta_f", [H], F32R, kind="ExternalInput")
    msk_d = nc.dram_tensor("mask_b", [S], F32, kind="ExternalInput")
    onr_d = nc.dram_tensor("ones_fr", [128, 128], F32R, kind="ExternalInput")
    onb_d = nc.dram_tensor("ones_bf", [128, 64], BF16, kind="ExternalInput")
    out_d = nc.dram_tensor("out_chunk", [4, 128, H], F32, kind="ExternalOutput")

    with tile.TileContext(nc) as tc:
        _body(tc, hsT_d, hso_d, wq_d, wk_d, wv_d, wd_d, bq_d, bk_d, bv_d,
              bd_d, gam_d, bet_d, msk_d, onr_d, onb_d, out_d)
    nc.compile()
    return nc


def _body(tc, hsT_d, hso_d, wq_d, wk_d, wv_d, wd_d, bq_d, bk_d, bv_d,
          bd_d, gam_d, bet_d, msk_d, onr_d, onb_d, out_d):
    nc = tc.nc
    Exp = mybir.ActivationFunctionType.Exp
    Sqrt = mybir.ActivationFunctionType.Sqrt
    add_op = mybir.AluOpType.add
    sub_op = mybir.AluOpType.subtract
    mul_op = mybir.AluOpType.mult

    # ---------------- persistent SBUF ----------------
    persist = tc.alloc_tile_pool(name="persist", bufs=1)
    qT_sb = persist.tile([128, 2, S], BF16)      # [dim-in-pair, pair, tok]
    kT_sb = persist.tile([128, 2, S], BF16)
    vaug_sb = persist.tile([128, 16, 4, 65], BF16)  # [key-in-tile, keytile, head, 64v+1]
    ctxT_sb = persist.tile([128, 2, S], BF16)    # normalized ctx, feature-major
    cxo_sb = persist.tile([64, 2, S], BF16)      # odd heads before partition shift
    wd_sb = persist.tile([128, 2, H], BF16)
    bq_sb = persist.tile([128, 2], F32)
    bk_sb = persist.tile([128, 2], F32)
    msk_sb = persist.tile([128, 16], F32)
    onr_sb = persist.tile([128, 128], F32R)
    bv_sb = persist.tile([1, 256], F32R)
    bd_sb = persist.tile([1, H], F32R)
    gam_sb = persist.tile([1, H], F32R)
    bet_sb = persist.tile([1, H], F32R)
    eps_sb = persist.tile([128, 1], F32)

    nc.vector.memset(eps_sb, EPS)
    nc.sync.dma_start(out=onr_sb, in_=onr_d.ap())
    # per-head ones column of v~ (bf16 ones straight from DRAM)
    nc.sync.dma_start(
        out=vaug_sb[:, :, :, 64:65],
        in_=onb_d.ap().rearrange("p (a b) -> p a b", a=16).unsqueeze(3),
    )

    nc.sync.dma_start(out=bq_sb, in_=bq_d.ap().rearrange("(c p) -> p c", p=128))
    nc.sync.dma_start(out=bk_sb, in_=bk_d.ap().rearrange("(c p) -> p c", p=128))
    nc.sync.dma_start(out=bv_sb, in_=bv_d.ap().unsqueeze(0))
    nc.sync.dma_start(out=bd_sb, in_=bd_d.ap().unsqueeze(0))
    nc.sync.dma_start(out=gam_sb, in_=gam_d.ap().unsqueeze(0))
    nc.sync.dma_start(out=bet_sb, in_=bet_d.ap().unsqueeze(0))
    nc.sync.dma_start(out=msk_sb, in_=msk_d.ap().rearrange("(t p) -> p t", p=128))
    nc.sync.dma_start(out=wd_sb, in_=wd_d.ap().rearrange("(c p) d -> p c d", p=128))

    # ---------------- load pool (released after QKV) ----------------
    load = tc.alloc_tile_pool(name="load", bufs=1)
    hsT_sb = load.tile([128, 8, S], BF16)        # [feat-in-chunk, featchunk, tok]
    wq_sb = load.tile([128, 8, 256], BF16)
    wk_sb = load.tile([128, 8, 256], BF16)
    wv_sb = load.tile([128, 8, 256], BF16)

    nc.sync.dma_start(out=wq_sb, in_=wq_d.ap().rearrange("(c p) d -> p c d", p=128))
    nc.sync.dma_start(out=wk_sb, in_=wk_d.ap().rearrange("(c p) d -> p c d", p=128))
    nc.sync.dma_start(out=wv_sb, in_=wv_d.ap().rearrange("(c p) d -> p c d", p=128))
    # hidden states arrive in token-chunk slices so compute can start early
    hsT_src = hsT_d.ap().rearrange("(c p) (q w) -> q p c w", p=128, w=512)
    for t4 in range(4):
        nc.sync.dma_start(out=hsT_sb[:, :, ts(t4, 512)], in_=hsT_src[t4])

    # ---------------- QKV projections ----------------
    qkv_ps = tc.alloc_tile_pool(name="qkv_ps", bufs=3, space="PSUM")

    for t4 in range(4):
        for pr in range(2):
            for w_sb, b_sb, o_sb in (
                (wq_sb, bq_sb, qT_sb),
                (wk_sb, bk_sb, kT_sb),
            ):
                ps = qkv_ps.tile([128, 512], F32, tag="qk_ps")
                for kc in range(8):
                    nc.tensor.matmul(
                        ps,
                        lhsT=w_sb[:, kc, ts(pr, 128)],
                        rhs=hsT_sb[:, kc, ts(t4, 512)],
                        start=(kc == 0),
                        stop=(kc == 7),
                    )
                nc.vector.tensor_scalar_add(
                    out=o_sb[:, pr, ts(t4, 512)], in0=ps,
                    scalar1=b_sb[:, pr:pr + 1],
                )
        for tt in range(4):          # token tiles of 128 within this 512 chunk
            t16 = 4 * t4 + tt
            ps = qkv_ps.tile([128, 256], F32, tag="v_ps")
            for kc in range(8):
                nc.tensor.matmul(
                    ps,
                    lhsT=hsT_sb[:, kc, ts(t16, 128)],
                    rhs=wv_sb[:, kc, :],
                    start=(kc == 0),
                    stop=False,
                )
            nc.tensor.matmul(        # + bv broadcast over tokens (K=1, f32r)
                ps,
                lhsT=onr_sb[0:1, 0:128],
                rhs=bv_sb[0:1, :],
                start=False,
                stop=True,
            )
            nc.vector.tensor_copy(
                out=vaug_sb[:, t16, :, 0:64],
                in_=ps.rearrange("p (h d) -> p h d", h=4),
            )

    qkv_ps.release()
    load.release()

    # ------- attention -> dense -> chunked ReduceScatter -> LayerNorm -------
    dram = tc.alloc_tile_pool(name="dram", bufs=1, space="DRAM")
    rs_in = dram.tile([4, CHUNK, H], F32)        # per-qc partial projections
    rs_out = dram.tile([4, 128, H], F32)         # per-qc owned token rows

    at_sc = tc.alloc_tile_pool(name="at_sc", bufs=2, space="PSUM")    # 4 banks
    at_ctx = tc.alloc_tile_pool(name="at_ctx", bufs=1, space="PSUM")  # 2 banks
    at_rbc = tc.alloc_tile_pool(name="at_rbc", bufs=2, space="PSUM")  # 2 banks
    at_sb = tc.alloc_tile_pool(name="at_sb", bufs=3)
    fin_sb = tc.alloc_tile_pool(name="fin_sb", bufs=2)

    hso_sb = fin_sb.tile([128, 4, H], F32, bufs=1)
    nc.sync.dma_start(out=hso_sb, in_=hso_d.ap().rearrange("q p d -> p q d"))

    # broadcast bd / gamma / beta across partitions via K=1 matmuls (f32r)
    bdb_sb = fin_sb.tile([128, H], F32, bufs=1)
    gmb_sb = fin_sb.tile([128, H], F32, bufs=1)
    btb_sb = fin_sb.tile([128, H], F32, bufs=1)
    for src, dst in ((bd_sb, bdb_sb), (gam_sb, gmb_sb), (bet_sb, btb_sb)):
        for nh in range(2):
            pb = at_rbc.tile([128, 512], F32, tag="rbc")
            nc.tensor.matmul(
                pb,
                lhsT=onr_sb[0:1, 0:128],
                rhs=src[0:1, ts(nh, 512)],
            )
            nc.vector.tensor_copy(out=dst[:, ts(nh, 512)], in_=pb)

    out_ap = out_d.ap()
    for qc in range(4):
        for pr in range(2):
            ctx_e = at_ctx.tile([65, 512], F32, tag="ctx_e")
            ctx_o = at_ctx.tile([65, 512], F32, tag="ctx_o")
            for kt in range(16):
                sc = at_sc.tile([128, 1024], F32, tag="sc")
                nc.tensor.matmul(
                    sc[:, 0:512],
                    lhsT=kT_sb[0:64, pr, ts(kt, 128)],
                    rhs=qT_sb[0:64, pr, ts(qc, 512)],
                )
                nc.tensor.matmul(
                    sc[:, 512:1024],
                    lhsT=kT_sb[64:128, pr, ts(kt, 128)],
                    rhs=qT_sb[64:128, pr, ts(qc, 512)],
                )
                ex = at_sb.tile([128, 1024], BF16, tag="ex")
                nc.scalar.activation(
                    out=ex, in_=sc[:, :], func=Exp,
                    bias=msk_sb[:, kt:kt + 1], scale=0.125,
                )
                nc.tensor.matmul(
                    ctx_e,
                    lhsT=vaug_sb[:, kt, 2 * pr, :],
                    rhs=ex[:, 0:512],
                    start=(kt == 0), stop=(kt == 15),
                )
                nc.tensor.matmul(
                    ctx_o,
                    lhsT=vaug_sb[:, kt, 2 * pr + 1, :],
                    rhs=ex[:, 512:1024],
                    start=(kt == 0), stop=(kt == 15),
                )
            # merged softmax denominators for both heads: one reciprocal
            lrec = at_sb.tile([128, 1024], F32, tag="lrec")
            nc.vector.tensor_copy(out=lrec[64:65, 0:512], in_=ctx_e[64:65, :])
            nc.vector.tensor_copy(out=lrec[64:65, 512:1024], in_=ctx_o[64:65, :])
            nc.vector.reciprocal(out=lrec[64:65, :], in_=lrec[64:65, :])
            lrecr = at_sb.tile([128, 1024], F32R, tag="lrecr")
            nc.vector.tensor_copy(out=lrecr[64:65, :], in_=lrec[64:65, :])
            for hodd, ctx_ps in ((0, ctx_e), (1, ctx_o)):
                rbc = at_rbc.tile([128, 512], F32, tag="rbc")
                nc.tensor.matmul(
                    rbc[0:64, :],
                    lhsT=onr_sb[64:65, 0:64],
                    rhs=lrecr[64:65, ts(hodd, 512)],
                    tile_position=(64, 0),
                )
                rbs = at_sb.tile([64, 512], F32, tag="rbs")
                nc.vector.tensor_copy(out=rbs, in_=rbc[0:64, :])
                if hodd == 0:
                    dst = ctxT_sb[0:64, pr, ts(qc, 512)]
                else:
                    dst = cxo_sb[0:64, pr, ts(qc, 512)]
                nc.vector.tensor_tensor(
                    out=dst, in0=ctx_ps[0:64, :], in1=rbs, op=mul_op,
                )
            # odd-head rows into ctxT partitions 64..127 (only DMA can
            # cross partitions)
            nc.sync.dma_start(
                out=ctxT_sb[64:128, pr, ts(qc, 512)],
                in_=cxo_sb[0:64, pr, ts(qc, 512)],
            )

        # ---- partial dense for this chunk -> DRAM -> ReduceScatter ----
        for tt in range(4):
            pj = fin_sb.tile([128, H], F32, tag="pj")
            for nh in range(2):
                ps = at_rbc.tile([128, 512], F32, tag="rbc")
                for pr in range(2):
                    nc.tensor.matmul(
                        ps,
                        lhsT=ctxT_sb[:, pr, ts(4 * qc + tt, 128)],
                        rhs=wd_sb[:, pr, ts(nh, 512)],
                        start=(pr == 0),
                        stop=(pr == 1),
                    )
                nc.vector.tensor_copy(out=pj[:, ts(nh, 512)], in_=ps)
            nc.sync.dma_start(out=rs_in[qc, ts(tt, 128), :], in_=pj)

        nc.gpsimd.collective_compute(
            "ReduceScatter",
            add_op,
            replica_groups=GROUPS,
            ins=[rs_in[qc].flatten()],
            outs=[rs_out[qc].flatten()],
        )

        # ---- residual + bias + LayerNorm on the owned 128 rows ----
        x = fin_sb.tile([128, H], F32, tag="x")
        nc.sync.dma_start(out=x, in_=rs_out[qc])
        nc.vector.tensor_tensor(out=x, in0=x, in1=hso_sb[:, qc, :], op=add_op)
        nc.vector.tensor_tensor(out=x, in0=x, in1=bdb_sb, op=add_op)
        stats = fin_sb.tile([128, 2, 6], F32, tag="stats")
        for i in range(2):
            nc.vector.bn_stats(out=stats[:, i, :], in_=x[:, ts(i, 512)])
        mv = fin_sb.tile([128, 2], F32, tag="mv")
        nc.vector.bn_aggr(out=mv, in_=stats)
        sd = fin_sb.tile([128, 1], F32, tag="sd")
        nc.scalar.activation(out=sd, in_=mv[:, 1:2], func=Sqrt, bias=eps_sb, scale=1.0)
        rinv = fin_sb.tile([128, 1], F32, tag="rinv")
        nc.vector.reciprocal(out=rinv, in_=sd)
        nc.vector.tensor_scalar(
            out=x, in0=x, scalar1=mv[:, 0:1], scalar2=rinv,
            op0=sub_op, op1=mul_op,
        )
        nc.vector.tensor_tensor(out=x, in0=x, in1=gmb_sb, op=mul_op)
        nc.vector.tensor_tensor(out=x, in0=x, in1=btb_sb, op=add_op)
        nc.sync.dma_start(out=out_ap[qc], in_=x)

    fin_sb.release()
    at_sb.release()
    at_rbc.release()
    at_ctx.release()
    at_sc.release()
    dram.release()
    persist.release()


_NC_CACHE = {}


def _get_nc():
    if "nc" not in _NC_CACHE:
        _NC_CACHE["nc"] = build_nc()
    return _NC_CACHE["nc"]


def _bf16(x):
    import ml_dtypes
    return np.ascontiguousarray(x.astype(ml_dtypes.bfloat16))


def shard_inputs(inputs):
    import ml_dtypes
    hs = np.ascontiguousarray(np.asarray(inputs["hidden_states"], dtype=np.float32))
    mask = np.asarray(inputs["attention_mask"], dtype=np.float32)
    Wq = np.asarray(inputs["Wq"], dtype=np.float32)
    Wk = np.asarray(inputs["Wk"], dtype=np.float32)
    Wv = np.asarray(inputs["Wv"], dtype=np.float32)
    Wd = np.asarray(inputs["Wd"], dtype=np.float32)
    bq = np.asarray(inputs["bq"], dtype=np.float32)
    bk = np.asarray(inputs["bk"], dtype=np.float32)
    bv = np.asarray(inputs["bv"], dtype=np.float32)
    bd = np.ascontiguousarray(np.asarray(inputs["bd"], dtype=np.float32))
    gam = np.ascontiguousarray(np.asarray(inputs["ln_gamma"], dtype=np.float32))
    bet = np.ascontiguousarray(np.asarray(inputs["ln_beta"], dtype=np.float32))

    hsT = [_bf16(hs[b].T) for b in range(B)]
    mask_b = [np.ascontiguousarray(mask[b, 0, 0, :]) for b in range(B)]
    ones_fr = np.ones((128, 128), np.float32)
    ones_bf = np.ones((128, 64), ml_dtypes.bfloat16)

    in_maps = []
    for c in range(NCORES):
        b, g = c // 4, c % 4
        sl = slice(256 * g, 256 * g + 256)
        # owned token rows: 512*qc + 128*g .. +128 for qc in 0..3
        hs_own = np.stack(
            [hs[b, 512 * qc + 128 * g: 512 * qc + 128 * g + 128] for qc in range(4)]
        )
        in_maps.append({
            "hsT": hsT[b],
            "hs_own": np.ascontiguousarray(hs_own),
            "wqT": _bf16(Wq[sl, :].T),
            "wkT": _bf16(Wk[sl, :].T),
            "wvT": _bf16(Wv[sl, :].T),
            "wdT": _bf16(Wd[:, sl].T),
            "bq_s": np.ascontiguousarray(bq[sl]),
            "bk_s": np.ascontiguousarray(bk[sl]),
            "bv_s": np.ascontiguousarray(bv[sl]),
            "bd_f": bd,
            "gamma_f": gam,
            "beta_f": bet,
            "mask_b": mask_b[b],
            "ones_fr": ones_fr,
            "ones_bf": ones_bf,
        })
    return in_maps


def assemble(results):
    out = np.zeros((B, S, H), np.float32)
    for c in range(NCORES):
        b, g = c // 4, c % 4
        for qc in range(4):
            r0 = 512 * qc + 128 * g
            out[b, r0:r0 + 128, :] = results[c]["out_chunk"][qc]
    return out


LAST_RESULT = None


def kernel(**inputs):
    global LAST_RESULT
    from concourse.bass_utils import run_bass_kernel_spmd

    nc = _get_nc()
    in_maps = shard_inputs(inputs)
    trace = bool(int(os.environ.get("KERNEL_TRACE", "0")))
    res = run_bass_kernel_spmd(nc, in_maps, list(range(NCORES)), trace=trace)
    LAST_RESULT = res
    return assemble(res.results)


def simulate(inputs):
    """CoreSim-based check (no hardware)."""
    from concourse.bass_interp import MultiCoreSim

    nc = _get_nc()
    in_maps = shard_inputs(inputs)
    sim = MultiCoreSim(nc, NCORES)
    for c in range(NCORES):
        for k, v in in_maps[c].items():
            sim.cores[c].tensor(k)[:] = v
    sim.simulate(check_with_hw=False)
    results = [{"out_chunk": np.array(sim.cores[c].tensor("out_chunk"))}
               for c in range(NCORES)]
    return assemble(results)


# revision 15
# speedup vs baseline: 1.4444x; 1.0157x over previous
"""Trainium2 Bass kernel for ALBERT attention (B=2, S=2048, H=1024, NH=16).

Sharding over 8 NeuronCores: 2 batches x 4 head-groups (tensor parallel over
heads within each batch).  Core c handles batch b = c//4 and heads
[4g, 4g+4) where g = c%4.  The kernel pipelines over four 512-token chunks:
for each chunk it runs attention (both head pairs), the partial output
projection, and a ReduceScatter(add) over the batch's 4-core group that both
sums the head-group partials and scatters token ownership; the RS of chunk i
overlaps the attention of chunk i+1.  Core (b, g) ends up owning token rows
512*qc + 128*g .. +128 for qc in 0..3, applies residual + bias + LayerNorm,
and writes those four [128, 1024] slices.

Matmuls run in bf16 (inputs host-cast; fp32 PSUM accumulation).  The softmax
normalization (1/l) and the tiny K=1 broadcast matmuls stay float32r.

Per-core dataflow:
  hsT [1024, 2048] bf16   (host-pretransposed hidden states of its batch)
  qT/kT = W.T-slices @ hsT          (feature-major, head pairs stacked 64+64)
  v     = hsT.T @ WvT-slice         (token-major) + ones column per head
  per 512-token q chunk, per head pair, per 128-key tile:
      scoresT[key, q] = k @ qT   (two heads row-packed, K=64)
      expT = exp(0.125*scoresT + mask[key])   (ScalarE, mask as bias)
      ctx~T[65, q] += [v | 1].T @ expT        (PSUM-accumulated over keys)
  ctxT_h = ctx~T[:64] / ctx~T[64]   (merged reciprocal + K=1 bcast matmul)
  partial(qc) = ctxT(qc).T @ WdT-slice -> ReduceScatter(qc) over group of 4
  out = LN(rs + hs_own + bd) * gamma + beta
"""

import os
import sys

import numpy as np

for _p in ("/opt/trn_rl_repo",):
    if _p not in sys.path:
        sys.path.insert(0, _p)

import concourse.bass as bass
import concourse.mybir as mybir
import concourse.tile as tile
from concourse import bacc
from concourse.bass import ts

F32 = mybir.dt.float32
F32R = mybir.dt.float32r
BF16 = mybir.dt.bfloat16

H, NH, HD = 1024, 16, 64
B, S = 2, 2048
NCORES = 8
GROUPS = [[0, 1, 2, 3], [4, 5, 6, 7]]
CHUNK = 512          # tokens per pipelined chunk (and per-core output rows)
EPS = 1e-12


def build_nc():
    nc = bacc.Bacc(
        "TRN2",
        target_bir_lowering=False,
        debug=False,
        num_devices=NCORES,
    )

    hsT_d = nc.dram_tensor("hsT", [H, S], BF16, kind="ExternalInput")
    hso_d = nc.dram_tensor("hs_own", [4, 128, H], F32, kind="ExternalInput")
    wq_d = nc.dram_tensor("wqT", [H, 256], BF16, kind="ExternalInput")
    wk_d = nc.dram_tensor("wkT", [H, 256], BF16, kind="ExternalInput")
    wv_d = nc.dram_tensor("wvT", [H, 256], BF16, kind="ExternalInput")
    wd_d = nc.dram_tensor("wdT", [256, H], BF16, kind="ExternalInput")
    bq_d = nc.dram_tensor("bq_s", [256], F32, kind="ExternalInput")
    bk_d = nc.dram_tensor("bk_s", [256], F32, kind="ExternalInput")
    bv_d = nc.dram_tensor("bv_s", [256], F32R, kind="ExternalInput")
    bd_d = nc.dram_tensor("bd_f", [H], F32R, kind="ExternalInput")
    gam_d = nc.dram_tensor("gamma_f", [H], F32R, kind="ExternalInput")
    bet_d = nc.dram_tensor("beta_f", [H], F32R, kind="ExternalInput")
    msk_d = nc.dram_tensor("mask_b", [S], F32, kind="ExternalInput")
    onr_d = nc.dram_tensor("ones_fr", [128, 128], F32R, kind="ExternalInput")
    onb_d = nc.dram_tensor("ones_bf", [128, 64], BF16, kind="ExternalInput")
    out_d = nc.dram_tensor("out_chunk", [4, 128, H], F32, kind="ExternalOutput")

    with tile.TileContext(nc) as tc:
        _body(tc, hsT_d, hso_d, wq_d, wk_d, wv_d, wd_d, bq_d, bk_d, bv_d,
              bd_d, gam_d, bet_d, msk_d, onr_d, onb_d, out_d)
    nc.compile()
    return nc


def _body(tc, hsT_d, hso_d, wq_d, wk_d, wv_d, wd_d, bq_d, bk_d, bv_d,
          bd_d, gam_d, bet_d, msk_d, onr_d, onb_d, out_d):
    nc = tc.nc
    Exp = mybir.ActivationFunctionType.Exp
    Sqrt = mybir.ActivationFunctionType.Sqrt
    add_op = mybir.AluOpType.add
    sub_op = mybir.AluOpType.subtract
    mul_op = mybir.AluOpType.mult

    # ---------------- persistent SBUF ----------------
    persist = tc.alloc_tile_pool(name="persist", bufs=1)
    qT_sb = persist.tile([128, 2, S], BF16)      # [dim-in-pair, pair, tok]
    kT_sb = persist.tile([128, 2, S], BF16)
    vaug_sb = persist.tile([128, 16, 4, 65], BF16)  # [key-in-tile, keytile, head, 64v+1]
    ctxT_sb = persist.tile([128, 2, S], BF16)    # normalized ctx, feature-major
    cxo_sb = persist.tile([64, 2, S], BF16)      # odd heads before partition shift
    wd_sb = persist.tile([128, 2, H], BF16)
    bq_sb = persist.tile([128, 2], F32)
    bk_sb = persist.tile([128, 2], F32)
    msk_sb = persist.tile([128, 16], F32)
    onr_sb = persist.tile([128, 128], F32R)
    bv_sb = persist.tile([1, 256], F32R)
    bd_sb = persist.tile([1, H], F32R)
    gam_sb = persist.tile([1, H], F32R)
    bet_sb = persist.tile([1, H], F32R)
    eps_sb = persist.tile([128, 1], F32)

    nc.vector.memset(eps_sb, EPS)
    nc.sync.dma_start(out=onr_sb, in_=onr_d.ap())
    # per-head ones column of v~ (bf16 ones straight from DRAM)
    nc.sync.dma_start(
        out=vaug_sb[:, :, :, 64:65],
        in_=onb_d.ap().rearrange("p (a b) -> p a b", a=16).unsqueeze(3),
    )

    nc.sync.dma_start(out=bq_sb, in_=bq_d.ap().rearrange("(c p) -> p c", p=128))
    nc.sync.dma_start(out=bk_sb, in_=bk_d.ap().rearrange("(c p) -> p c", p=128))
    nc.sync.dma_start(out=bv_sb, in_=bv_d.ap().unsqueeze(0))
    nc.sync.dma_start(out=bd_sb, in_=bd_d.ap().unsqueeze(0))
    nc.sync.dma_start(out=gam_sb, in_=gam_d.ap().unsqueeze(0))
    nc.sync.dma_start(out=bet_sb, in_=bet_d.ap().unsqueeze(0))
    nc.sync.dma_start(out=msk_sb, in_=msk_d.ap().rearrange("(t p) -> p t", p=128))
    nc.sync.dma_start(out=wd_sb, in_=wd_d.ap().rearrange("(c p) d -> p c d", p=128))

    # ---------------- load pool (released after QKV) ----------------
    load = tc.alloc_tile_pool(name="load", bufs=1)
    hsT_sb = load.tile([128, 8, S], BF16)        # [feat-in-chunk, featchunk, tok]
    wq_sb = load.tile([128, 8, 256], BF16)
    wk_sb = load.tile([128, 8, 256], BF16)
    wv_sb = load.tile([128, 8, 256], BF16)

    nc.sync.dma_start(out=wq_sb, in_=wq_d.ap().rearrange("(c p) d -> p c d", p=128))
    nc.sync.dma_start(out=wk_sb, in_=wk_d.ap().rearrange("(c p) d -> p c d", p=128))
    nc.sync.dma_start(out=wv_sb, in_=wv_d.ap().rearrange("(c p) d -> p c d", p=128))
    # hidden states arrive in token-chunk slices so compute can start early
    hsT_src = hsT_d.ap().rearrange("(c p) (q w) -> q p c w", p=128, w=512)
    for t4 in range(4):
        nc.sync.dma_start(out=hsT_sb[:, :, ts(t4, 512)], in_=hsT_src[t4])

    # ---------------- QKV projections ----------------
    qkv_ps = tc.alloc_tile_pool(name="qkv_ps", bufs=3, space="PSUM")

    for t4 in range(4):
        for pr in range(2):
            for w_sb, b_sb, o_sb in (
                (wq_sb, bq_sb, qT_sb),
                (wk_sb, bk_sb, kT_sb),
            ):
                ps = qkv_ps.tile([128, 512], F32, tag="qk_ps")
                for kc in range(8):
                    nc.tensor.matmul(
                        ps,
                        lhsT=w_sb[:, kc, ts(pr, 128)],
                        rhs=hsT_sb[:, kc, ts(t4, 512)],
                        start=(kc == 0),
                        stop=(kc == 7),
                    )
                nc.vector.tensor_scalar_add(
                    out=o_sb[:, pr, ts(t4, 512)], in0=ps,
                    scalar1=b_sb[:, pr:pr + 1],
                )
        for tt in range(4):          # token tiles of 128 within this 512 chunk
            t16 = 4 * t4 + tt
            ps = qkv_ps.tile([128, 256], F32, tag="v_ps")
            for kc in range(8):
                nc.tensor.matmul(
                    ps,
                    lhsT=hsT_sb[:, kc, ts(t16, 128)],
                    rhs=wv_sb[:, kc, :],
                    start=(kc == 0),
                    stop=False,
                )
            nc.tensor.matmul(        # + bv broadcast over tokens (K=1, f32r)
                ps,
                lhsT=onr_sb[0:1, 0:128],
                rhs=bv_sb[0:1, :],
                start=False,
                stop=True,
            )
            nc.vector.tensor_copy(
                out=vaug_sb[:, t16, :, 0:64],
                in_=ps.rearrange("p (h d) -> p h d", h=4),
            )

    qkv_ps.release()
    load.release()

    # ------- attention -> dense -> chunked ReduceScatter -> LayerNorm -------
    dram = tc.alloc_tile_pool(name="dram", bufs=1, space="DRAM")
    rs_in = dram.tile([4, CHUNK, H], F32)        # per-qc partial projections
    rs_out = dram.tile([4, 128, H], F32)         # per-qc owned token rows

    at_sc = tc.alloc_tile_pool(name="at_sc", bufs=2, space="PSUM")    # 4 banks
    at_ctx = tc.alloc_tile_pool(name="at_ctx", bufs=1, space="PSUM")  # 2 banks
    at_rbc = tc.alloc_tile_pool(name="at_rbc", bufs=2, space="PSUM")  # 2 banks
    at_sb = tc.alloc_tile_pool(name="at_sb", bufs=3)
    fin_sb = tc.alloc_tile_pool(name="fin_sb", bufs=2)

    hso_sb = fin_sb.tile([128, 4, H], F32, bufs=1)
    nc.sync.dma_start(out=hso_sb, in_=hso_d.ap().rearrange("q p d -> p q d"))

    # broadcast bd / gamma / beta across partitions via K=1 matmuls (f32r)
    bdb_sb = fin_sb.tile([128, H], F32, bufs=1)
    gmb_sb = fin_sb.tile([128, H], F32, bufs=1)
    btb_sb = fin_sb.tile([128, H], F32, bufs=1)
    for src, dst in ((bd_sb, bdb_sb), (gam_sb, gmb_sb), (bet_sb, btb_sb)):
        for nh in range(2):
            pb = at_rbc.tile([128, 512], F32, tag="rbc")
            nc.tensor.matmul(
                pb,
                lhsT=onr_sb[0:1, 0:128],
                rhs=src[0:1, ts(nh, 512)],
            )
            nc.vector.tensor_copy(out=dst[:, ts(nh, 512)], in_=pb)

    out_ap = out_d.ap()
    for qc in range(4):
        for pr in range(2):
            ctx_e = at_ctx.tile([65, 512], F32, tag="ctx_e")
            ctx_o = at_ctx.tile([65, 512], F32, tag="ctx_o")

            # software-pipelined: emit ctx(kt-1) after scores(kt) so the PE's
            # in-order stream never stalls on the exp of the current tile
            def emit_ctx(kt, ex):
                nc.tensor.matmul(
                    ctx_e,
                    lhsT=vaug_sb[:, kt, 2 * pr, :],
                    rhs=ex[:, 0:512],
                    start=(kt == 0), stop=(kt == 15),
                )
                nc.tensor.matmul(
                    ctx_o,
                    lhsT=vaug_sb[:, kt, 2 * pr + 1, :],
                    rhs=ex[:, 512:1024],
                    start=(kt == 0), stop=(kt == 15),
                )

            prev = None
            for kt in range(16):
                sc = at_sc.tile([128, 1024], F32, tag="sc")
                nc.tensor.matmul(
                    sc[:, 0:512],
                    lhsT=kT_sb[0:64, pr, ts(kt, 128)],
                    rhs=qT_sb[0:64, pr, ts(qc, 512)],
                )
                nc.tensor.matmul(
                    sc[:, 512:1024],
                    lhsT=kT_sb[64:128, pr, ts(kt, 128)],
                    rhs=qT_sb[64:128, pr, ts(qc, 512)],
                )
                ex = at_sb.tile([128, 1024], BF16, tag="ex")
                nc.scalar.activation(
                    out=ex, in_=sc[:, :], func=Exp,
                    bias=msk_sb[:, kt:kt + 1], scale=0.125,
                )
                if prev is not None:
                    emit_ctx(*prev)
                prev = (kt, ex)
            emit_ctx(*prev)
            # merged softmax denominators for both heads: one reciprocal
            lrec = at_sb.tile([128, 1024], F32, tag="lrec")
            nc.vector.tensor_copy(out=lrec[64:65, 0:512], in_=ctx_e[64:65, :])
            nc.vector.tensor_copy(out=lrec[64:65, 512:1024], in_=ctx_o[64:65, :])
            nc.vector.reciprocal(out=lrec[64:65, :], in_=lrec[64:65, :])
            lrecr = at_sb.tile([128, 1024], F32R, tag="lrecr")
            nc.vector.tensor_copy(out=lrecr[64:65, :], in_=lrec[64:65, :])
            for hodd, ctx_ps in ((0, ctx_e), (1, ctx_o)):
                rbc = at_rbc.tile([128, 512], F32, tag="rbc")
                nc.tensor.matmul(
                    rbc[0:64, :],
                    lhsT=onr_sb[64:65, 0:64],
                    rhs=lrecr[64:65, ts(hodd, 512)],
                    tile_position=(64, 0),
                )
                rbs = at_sb.tile([64, 512], F32, tag="rbs")
                nc.vector.tensor_copy(out=rbs, in_=rbc[0:64, :])
                if hodd == 0:
                    dst = ctxT_sb[0:64, pr, ts(qc, 512)]
                else:
                    dst = cxo_sb[0:64, pr, ts(qc, 512)]
                nc.vector.tensor_tensor(
                    out=dst, in0=ctx_ps[0:64, :], in1=rbs, op=mul_op,
                )
            # odd-head rows into ctxT partitions 64..127 (only DMA can
            # cross partitions)
            nc.sync.dma_start(
                out=ctxT_sb[64:128, pr, ts(qc, 512)],
                in_=cxo_sb[0:64, pr, ts(qc, 512)],
            )

        # ---- partial dense for this chunk -> DRAM -> ReduceScatter ----
        for tt in range(4):
            pj = fin_sb.tile([128, H], F32, tag="pj")
            for nh in range(2):
                ps = at_rbc.tile([128, 512], F32, tag="rbc")
                for pr in range(2):
                    nc.tensor.matmul(
                        ps,
                        lhsT=ctxT_sb[:, pr, ts(4 * qc + tt, 128)],
                        rhs=wd_sb[:, pr, ts(nh, 512)],
                        start=(pr == 0),
                        stop=(pr == 1),
                    )
                nc.vector.tensor_copy(out=pj[:, ts(nh, 512)], in_=ps)
            nc.sync.dma_start(out=rs_in[qc, ts(tt, 128), :], in_=pj)

        nc.gpsimd.collective_compute(
            "ReduceScatter",
            add_op,
            replica_groups=GROUPS,
            ins=[rs_in[qc].flatten()],
            outs=[rs_out[qc].flatten()],
        )

        # ---- residual + bias + LayerNorm on the owned 128 rows ----
        x = fin_sb.tile([128, H], F32, tag="x")
        nc.sync.dma_start(out=x, in_=rs_out[qc])
        nc.vector.tensor_tensor(out=x, in0=x, in1=hso_sb[:, qc, :], op=add_op)
        nc.vector.tensor_tensor(out=x, in0=x, in1=bdb_sb, op=add_op)
        stats = fin_sb.tile([128, 2, 6], F32, tag="stats")
        for i in range(2):
            nc.vector.bn_stats(out=stats[:, i, :], in_=x[:, ts(i, 512)])
        mv = fin_sb.tile([128, 2], F32, tag="mv")
        nc.vector.bn_aggr(out=mv, in_=stats)
        sd = fin_sb.tile([128, 1], F32, tag="sd")
        nc.scalar.activation(out=sd, in_=mv[:, 1:2], func=Sqrt, bias=eps_sb, scale=1.0)
        rinv = fin_sb.tile([128, 1], F32, tag="rinv")
        nc.vector.reciprocal(out=rinv, in_=sd)
        nc.vector.tensor_scalar(
            out=x, in0=x, scalar1=mv[:, 0:1], scalar2=rinv,
            op0=sub_op, op1=mul_op,
        )
        nc.vector.tensor_tensor(out=x, in0=x, in1=gmb_sb, op=mul_op)
        nc.vector.tensor_tensor(out=x, in0=x, in1=btb_sb, op=add_op)
        nc.sync.dma_start(out=out_ap[qc], in_=x)

    fin_sb.release()
    at_sb.release()
    at_rbc.release()
    at_ctx.release()
    at_sc.release()
    dram.release()
    persist.release()


_NC_CACHE = {}


def _get_nc():
    if "nc" not in _NC_CACHE:
        _NC_CACHE["nc"] = build_nc()
    return _NC_CACHE["nc"]


def _bf16(x):
    import ml_dtypes
    return np.ascontiguousarray(x.astype(ml_dtypes.bfloat16))


def shard_inputs(inputs):
    import ml_dtypes
    hs = np.ascontiguousarray(np.asarray(inputs["hidden_states"], dtype=np.float32))
    mask = np.asarray(inputs["attention_mask"], dtype=np.float32)
    Wq = np.asarray(inputs["Wq"], dtype=np.float32)
    Wk = np.asarray(inputs["Wk"], dtype=np.float32)
    Wv = np.asarray(inputs["Wv"], dtype=np.float32)
    Wd = np.asarray(inputs["Wd"], dtype=np.float32)
    bq = np.asarray(inputs["bq"], dtype=np.float32)
    bk = np.asarray(inputs["bk"], dtype=np.float32)
    bv = np.asarray(inputs["bv"], dtype=np.float32)
    bd = np.ascontiguousarray(np.asarray(inputs["bd"], dtype=np.float32))
    gam = np.ascontiguousarray(np.asarray(inputs["ln_gamma"], dtype=np.float32))
    bet = np.ascontiguousarray(np.asarray(inputs["ln_beta"], dtype=np.float32))

    hsT = [_bf16(hs[b].T) for b in range(B)]
    mask_b = [np.ascontiguousarray(mask[b, 0, 0, :]) for b in range(B)]
    ones_fr = np.ones((128, 128), np.float32)
    ones_bf = np.ones((128, 64), ml_dtypes.bfloat16)

    in_maps = []
    for c in range(NCORES):
        b, g = c // 4, c % 4
        sl = slice(256 * g, 256 * g + 256)
        # owned token rows: 512*qc + 128*g .. +128 for qc in 0..3
        hs_own = np.stack(
            [hs[b, 512 * qc + 128 * g: 512 * qc + 128 * g + 128] for qc in range(4)]
        )
        in_maps.append({
            "hsT": hsT[b],
            "hs_own": np.ascontiguousarray(hs_own),
            "wqT": _bf16(Wq[sl, :].T),
            "wkT": _bf16(Wk[sl, :].T),
            "wvT": _bf16(Wv[sl, :].T),
            "wdT": _bf16(Wd[:, sl].T),
            "bq_s": np.ascontiguousarray(bq[sl]),
            "bk_s": np.ascontiguousarray(bk[sl]),
            "bv_s": np.ascontiguousarray(bv[sl]),
            "bd_f": bd,
            "gamma_f": gam,
            "beta_f": bet,
            "mask_b": mask_b[b],
            "ones_fr": ones_fr,
            "ones_bf": ones_bf,
        })
    return in_maps


def assemble(results):
    out = np.zeros((B, S, H), np.float32)
    for c in range(NCORES):
        b, g = c // 4, c % 4
        for qc in range(4):
            r0 = 512 * qc + 128 * g
            out[b, r0:r0 + 128, :] = results[c]["out_chunk"][qc]
    return out


LAST_RESULT = None


def kernel(**inputs):
    global LAST_RESULT
    from concourse.bass_utils import run_bass_kernel_spmd

    nc = _get_nc()
    in_maps = shard_inputs(inputs)
    trace = bool(int(os.environ.get("KERNEL_TRACE", "0")))
    res = run_bass_kernel_spmd(nc, in_maps, list(range(NCORES)), trace=trace)
    LAST_RESULT = res
    return assemble(res.results)


def simulate(inputs):
    """CoreSim-based check (no hardware)."""
    from concourse.bass_interp import MultiCoreSim

    nc = _get_nc()
    in_maps = shard_inputs(inputs)
    sim = MultiCoreSim(nc, NCORES)
    for c in range(NCORES):
        for k, v in in_maps[c].items():
            sim.cores[c].tensor(k)[:] = v
    sim.simulate(check_with_hw=False)
    results = [{"out_chunk": np.array(sim.cores[c].tensor("out_chunk"))}
               for c in range(NCORES)]
    return assemble(results)


# revision 16
# speedup vs baseline: 1.4639x; 1.0135x over previous
"""Trainium2 Bass kernel for ALBERT attention (B=2, S=2048, H=1024, NH=16).

Sharding over 8 NeuronCores: 2 batches x 4 head-groups (tensor parallel over
heads within each batch).  Core c handles batch b = c//4 and heads
[4g, 4g+4) where g = c%4.  The kernel pipelines over four 512-token chunks:
for each chunk it runs attention (both head pairs), the partial output
projection, and a ReduceScatter(add) over the batch's 4-core group that both
sums the head-group partials and scatters token ownership; the RS of chunk i
overlaps the attention of chunk i+1.  Core (b, g) ends up owning token rows
512*qc + 128*g .. +128 for qc in 0..3, applies residual + bias + LayerNorm,
and writes those four [128, 1024] slices.

Matmuls run in bf16 (inputs host-cast; fp32 PSUM accumulation).  The softmax
normalization (1/l) and the tiny K=1 broadcast matmuls stay float32r.

Per-core dataflow:
  hsT [1024, 2048] bf16   (host-pretransposed hidden states of its batch)
  qT/kT = W.T-slices @ hsT          (feature-major, head pairs stacked 64+64)
  v     = hsT.T @ WvT-slice         (token-major) + ones column per head
  per 512-token q chunk, per head pair, per 128-key tile:
      scoresT[key, q] = k @ qT   (two heads row-packed, K=64)
      expT = exp(0.125*scoresT + mask[key])   (ScalarE, mask as bias)
      ctx~T[65, q] += [v | 1].T @ expT        (PSUM-accumulated over keys)
  ctxT_h = ctx~T[:64] / ctx~T[64]   (merged reciprocal + K=1 bcast matmul)
  partial(qc) = ctxT(qc).T @ WdT-slice -> ReduceScatter(qc) over group of 4
  out = LN(rs + hs_own + bd) * gamma + beta
"""

import os
import sys

import numpy as np

for _p in ("/opt/trn_rl_repo",):
    if _p not in sys.path:
        sys.path.insert(0, _p)

import concourse.bass as bass
import concourse.mybir as mybir
import concourse.tile as tile
from concourse import bacc
from concourse.bass import ts

F32 = mybir.dt.float32
F32R = mybir.dt.float32r
BF16 = mybir.dt.bfloat16

H, NH, HD = 1024, 16, 64
B, S = 2, 2048
NCORES = 8
GROUPS = [[0, 1, 2, 3], [4, 5, 6, 7]]
CHUNK = 512          # tokens per pipelined chunk (and per-core output rows)
EPS = 1e-12


def build_nc():
    nc = bacc.Bacc(
        "TRN2",
        target_bir_lowering=False,
        debug=False,
        num_devices=NCORES,
    )

    hsT_d = nc.dram_tensor("hsT", [H, S], BF16, kind="ExternalInput")
    hso_d = nc.dram_tensor("hs_own", [4, 128, H], F32, kind="ExternalInput")
    wq_d = nc.dram_tensor("wqT", [H, 256], BF16, kind="ExternalInput")
    wk_d = nc.dram_tensor("wkT", [H, 256], BF16, kind="ExternalInput")
    wv_d = nc.dram_tensor("wvT", [H, 256], BF16, kind="ExternalInput")
    wd_d = nc.dram_tensor("wdT", [256, H], BF16, kind="ExternalInput")
    bq_d = nc.dram_tensor("bq_s", [256], F32, kind="ExternalInput")
    bk_d = nc.dram_tensor("bk_s", [256], F32, kind="ExternalInput")
    bv_d = nc.dram_tensor("bv_s", [256], F32R, kind="ExternalInput")
    bd_d = nc.dram_tensor("bd_f", [H], F32R, kind="ExternalInput")
    gam_d = nc.dram_tensor("gamma_f", [H], F32R, kind="ExternalInput")
    bet_d = nc.dram_tensor("beta_f", [H], F32R, kind="ExternalInput")
    msk_d = nc.dram_tensor("mask_b", [S], F32, kind="ExternalInput")
    onr_d = nc.dram_tensor("ones_fr", [128, 128], F32R, kind="ExternalInput")
    onb_d = nc.dram_tensor("ones_bf", [128, 64], BF16, kind="ExternalInput")
    out_d = nc.dram_tensor("out_chunk", [4, 128, H], F32, kind="ExternalOutput")

    with tile.TileContext(nc) as tc:
        _body(tc, hsT_d, hso_d, wq_d, wk_d, wv_d, wd_d, bq_d, bk_d, bv_d,
              bd_d, gam_d, bet_d, msk_d, onr_d, onb_d, out_d)
    nc.compile()
    return nc


def _body(tc, hsT_d, hso_d, wq_d, wk_d, wv_d, wd_d, bq_d, bk_d, bv_d,
          bd_d, gam_d, bet_d, msk_d, onr_d, onb_d, out_d):
    nc = tc.nc
    Exp = mybir.ActivationFunctionType.Exp
    Sqrt = mybir.ActivationFunctionType.Sqrt
    add_op = mybir.AluOpType.add
    sub_op = mybir.AluOpType.subtract
    mul_op = mybir.AluOpType.mult

    # ---------------- persistent SBUF ----------------
    persist = tc.alloc_tile_pool(name="persist", bufs=1)
    qT_sb = persist.tile([128, 2, S], BF16)      # [dim-in-pair, pair, tok]
    kT_sb = persist.tile([128, 2, S], BF16)
    vaug_sb = persist.tile([128, 16, 4, 65], BF16)  # [key-in-tile, keytile, head, 64v+1]
    ctxT_sb = persist.tile([128, 2, S], BF16)    # normalized ctx, feature-major
    cxo_sb = persist.tile([64, 2, S], BF16)      # odd heads before partition shift
    wd_sb = persist.tile([128, 2, H], BF16)
    bq_sb = persist.tile([128, 2], F32)
    bk_sb = persist.tile([128, 2], F32)
    msk_sb = persist.tile([128, 16], F32)
    onr_sb = persist.tile([128, 128], F32R)
    bv_sb = persist.tile([1, 256], F32R)
    bd_sb = persist.tile([1, H], F32R)
    gam_sb = persist.tile([1, H], F32R)
    bet_sb = persist.tile([1, H], F32R)
    eps_sb = persist.tile([128, 1], F32)

    nc.vector.memset(eps_sb, EPS)
    nc.sync.dma_start(out=onr_sb, in_=onr_d.ap())
    # per-head ones column of v~ (bf16 ones straight from DRAM)
    nc.sync.dma_start(
        out=vaug_sb[:, :, :, 64:65],
        in_=onb_d.ap().rearrange("p (a b) -> p a b", a=16).unsqueeze(3),
    )

    nc.sync.dma_start(out=bq_sb, in_=bq_d.ap().rearrange("(c p) -> p c", p=128))
    nc.sync.dma_start(out=bk_sb, in_=bk_d.ap().rearrange("(c p) -> p c", p=128))
    nc.sync.dma_start(out=bv_sb, in_=bv_d.ap().unsqueeze(0))
    nc.sync.dma_start(out=bd_sb, in_=bd_d.ap().unsqueeze(0))
    nc.sync.dma_start(out=gam_sb, in_=gam_d.ap().unsqueeze(0))
    nc.sync.dma_start(out=bet_sb, in_=bet_d.ap().unsqueeze(0))
    nc.sync.dma_start(out=msk_sb, in_=msk_d.ap().rearrange("(t p) -> p t", p=128))
    nc.sync.dma_start(out=wd_sb, in_=wd_d.ap().rearrange("(c p) d -> p c d", p=128))

    # ---------------- load pool (released after QKV) ----------------
    load = tc.alloc_tile_pool(name="load", bufs=1)
    hsT_sb = load.tile([128, 8, S], BF16)        # [feat-in-chunk, featchunk, tok]
    wq_sb = load.tile([128, 8, 256], BF16)
    wk_sb = load.tile([128, 8, 256], BF16)
    wv_sb = load.tile([128, 8, 256], BF16)

    nc.sync.dma_start(out=wq_sb, in_=wq_d.ap().rearrange("(c p) d -> p c d", p=128))
    nc.sync.dma_start(out=wk_sb, in_=wk_d.ap().rearrange("(c p) d -> p c d", p=128))
    nc.sync.dma_start(out=wv_sb, in_=wv_d.ap().rearrange("(c p) d -> p c d", p=128))
    # hidden states arrive in token-chunk slices so compute can start early
    hsT_src = hsT_d.ap().rearrange("(c p) (q w) -> q p c w", p=128, w=512)
    for t4 in range(4):
        nc.sync.dma_start(out=hsT_sb[:, :, ts(t4, 512)], in_=hsT_src[t4])

    # ---------------- QKV projections ----------------
    qkv_ps = tc.alloc_tile_pool(name="qkv_ps", bufs=3, space="PSUM")

    for t4 in range(4):
        for pr in range(2):
            for w_sb, b_sb, o_sb in (
                (wq_sb, bq_sb, qT_sb),
                (wk_sb, bk_sb, kT_sb),
            ):
                ps = qkv_ps.tile([128, 512], F32, tag="qk_ps")
                for kc in range(8):
                    nc.tensor.matmul(
                        ps,
                        lhsT=w_sb[:, kc, ts(pr, 128)],
                        rhs=hsT_sb[:, kc, ts(t4, 512)],
                        start=(kc == 0),
                        stop=(kc == 7),
                    )
                nc.vector.tensor_scalar_add(
                    out=o_sb[:, pr, ts(t4, 512)], in0=ps,
                    scalar1=b_sb[:, pr:pr + 1],
                )
        for tt in range(4):          # token tiles of 128 within this 512 chunk
            t16 = 4 * t4 + tt
            ps = qkv_ps.tile([128, 256], F32, tag="v_ps")
            for kc in range(8):
                nc.tensor.matmul(
                    ps,
                    lhsT=hsT_sb[:, kc, ts(t16, 128)],
                    rhs=wv_sb[:, kc, :],
                    start=(kc == 0),
                    stop=False,
                )
            nc.tensor.matmul(        # + bv broadcast over tokens (K=1, f32r)
                ps,
                lhsT=onr_sb[0:1, 0:128],
                rhs=bv_sb[0:1, :],
                start=False,
                stop=True,
            )
            nc.vector.tensor_copy(
                out=vaug_sb[:, t16, :, 0:64],
                in_=ps.rearrange("p (h d) -> p h d", h=4),
            )

    qkv_ps.release()
    load.release()

    # ------- attention -> dense -> chunked ReduceScatter -> LayerNorm -------
    dram = tc.alloc_tile_pool(name="dram", bufs=1, space="DRAM")
    rs_in = dram.tile([4, CHUNK, H], F32)        # per-qc partial projections
    rs_out = dram.tile([4, 128, H], F32)         # per-qc owned token rows

    at_sc = tc.alloc_tile_pool(name="at_sc", bufs=2, space="PSUM")    # 4 banks
    at_ctx = tc.alloc_tile_pool(name="at_ctx", bufs=1, space="PSUM")  # 2 banks
    at_rbc = tc.alloc_tile_pool(name="at_rbc", bufs=2, space="PSUM")  # 2 banks
    at_sb = tc.alloc_tile_pool(name="at_sb", bufs=3)
    fin_sb = tc.alloc_tile_pool(name="fin_sb", bufs=2)

    hso_sb = fin_sb.tile([128, 4, H], F32, bufs=1)
    nc.sync.dma_start(out=hso_sb, in_=hso_d.ap().rearrange("q p d -> p q d"))

    # broadcast bd / gamma / beta across partitions via K=1 matmuls (f32r)
    bdb_sb = fin_sb.tile([128, H], F32, bufs=1)
    gmb_sb = fin_sb.tile([128, H], F32, bufs=1)
    btb_sb = fin_sb.tile([128, H], F32, bufs=1)
    for src, dst in ((bd_sb, bdb_sb), (gam_sb, gmb_sb), (bet_sb, btb_sb)):
        for nh in range(2):
            pb = at_rbc.tile([128, 512], F32, tag="rbc")
            nc.tensor.matmul(
                pb,
                lhsT=onr_sb[0:1, 0:128],
                rhs=src[0:1, ts(nh, 512)],
            )
            nc.vector.tensor_copy(out=dst[:, ts(nh, 512)], in_=pb)

    out_ap = out_d.ap()
    for qc in range(4):
        for pr in range(2):
            ctx_e = at_ctx.tile([65, 512], F32, tag="ctx_e")
            ctx_o = at_ctx.tile([65, 512], F32, tag="ctx_o")

            # software-pipelined: emit ctx(kt-1) after scores(kt) so the PE's
            # in-order stream never stalls on the exp of the current tile
            def emit_ctx(kt, ex):
                nc.tensor.matmul(
                    ctx_e,
                    lhsT=vaug_sb[:, kt, 2 * pr, :],
                    rhs=ex[:, 0:512],
                    start=(kt == 0), stop=(kt == 15),
                )
                nc.tensor.matmul(
                    ctx_o,
                    lhsT=vaug_sb[:, kt, 2 * pr + 1, :],
                    rhs=ex[:, 512:1024],
                    start=(kt == 0), stop=(kt == 15),
                )

            prev = None
            for kt in range(16):
                sc = at_sc.tile([128, 1024], F32, tag="sc")
                nc.tensor.matmul(
                    sc[:, 0:512],
                    lhsT=kT_sb[0:64, pr, ts(kt, 128)],
                    rhs=qT_sb[0:64, pr, ts(qc, 512)],
                )
                nc.tensor.matmul(
                    sc[:, 512:1024],
                    lhsT=kT_sb[64:128, pr, ts(kt, 128)],
                    rhs=qT_sb[64:128, pr, ts(qc, 512)],
                )
                ex = at_sb.tile([128, 1024], BF16, tag="ex")
                nc.scalar.activation(
                    out=ex, in_=sc[:, :], func=Exp,
                    bias=msk_sb[:, kt:kt + 1], scale=0.125,
                )
                if prev is not None:
                    emit_ctx(*prev)
                prev = (kt, ex)
            emit_ctx(*prev)
            # merged softmax denominators for both heads: one reciprocal
            lrec = at_sb.tile([128, 1024], F32, tag="lrec")
            nc.vector.tensor_copy(out=lrec[64:65, 0:512], in_=ctx_e[64:65, :])
            nc.vector.tensor_copy(out=lrec[64:65, 512:1024], in_=ctx_o[64:65, :])
            nc.vector.reciprocal(out=lrec[64:65, :], in_=lrec[64:65, :])
            lrecr = at_sb.tile([128, 1024], F32R, tag="lrecr")
            nc.vector.tensor_copy(out=lrecr[64:65, :], in_=lrec[64:65, :])
            for hodd, ctx_ps in ((0, ctx_e), (1, ctx_o)):
                rbc = at_rbc.tile([128, 512], F32, tag="rbc")
                nc.tensor.matmul(
                    rbc[0:64, :],
                    lhsT=onr_sb[64:65, 0:64],
                    rhs=lrecr[64:65, ts(hodd, 512)],
                    tile_position=(64, 0),
                )
                rbs = at_sb.tile([64, 512], F32, tag="rbs")
                nc.vector.tensor_copy(out=rbs, in_=rbc[0:64, :])
                if hodd == 0:
                    dst = ctxT_sb[0:64, pr, ts(qc, 512)]
                else:
                    dst = cxo_sb[0:64, pr, ts(qc, 512)]
                nc.vector.tensor_tensor(
                    out=dst, in0=ctx_ps[0:64, :], in1=rbs, op=mul_op,
                )
            # odd-head rows into ctxT partitions 64..127 (only DMA can
            # cross partitions)
            nc.sync.dma_start(
                out=ctxT_sb[64:128, pr, ts(qc, 512)],
                in_=cxo_sb[0:64, pr, ts(qc, 512)],
            )

        # ---- partial dense for this chunk -> DRAM -> ReduceScatter ----
        for tt in range(4):
            pj = fin_sb.tile([128, H], F32, tag="pj")
            for nh in range(2):
                ps = at_rbc.tile([128, 512], F32, tag="rbc")
                for pr in range(2):
                    nc.tensor.matmul(
                        ps,
                        lhsT=ctxT_sb[:, pr, ts(4 * qc + tt, 128)],
                        rhs=wd_sb[:, pr, ts(nh, 512)],
                        start=(pr == 0),
                        stop=(pr == 1),
                    )
                nc.vector.tensor_copy(out=pj[:, ts(nh, 512)], in_=ps)
            nc.sync.dma_start(out=rs_in[qc, ts(tt, 128), :], in_=pj)

        nc.gpsimd.collective_compute(
            "ReduceScatter",
            add_op,
            replica_groups=GROUPS,
            ins=[rs_in[qc].flatten()],
            outs=[rs_out[qc].flatten()],
        )

    # ---- residual + bias + LayerNorm, off the attention/RS critical path.
    # rstd = exp(-0.5*ln(var+eps)) keeps all ScalarE work in the
    # natural_log_exp table set (no reloads between exp batches).
    Ln = mybir.ActivationFunctionType.Ln
    for qc in range(4):
        x = fin_sb.tile([128, H], F32, tag="x")
        nc.sync.dma_start(out=x, in_=rs_out[qc])
        nc.vector.tensor_tensor(out=x, in0=x, in1=hso_sb[:, qc, :], op=add_op)
        nc.vector.tensor_tensor(out=x, in0=x, in1=bdb_sb, op=add_op)
        stats = fin_sb.tile([128, 2, 6], F32, tag="stats")
        for i in range(2):
            nc.vector.bn_stats(out=stats[:, i, :], in_=x[:, ts(i, 512)])
        mv = fin_sb.tile([128, 2], F32, tag="mv")
        nc.vector.bn_aggr(out=mv, in_=stats)
        lv = fin_sb.tile([128, 1], F32, tag="lv")
        nc.scalar.activation(out=lv, in_=mv[:, 1:2], func=Ln, bias=eps_sb, scale=1.0)
        rinv = fin_sb.tile([128, 1], F32, tag="rinv")
        nc.scalar.activation(out=rinv, in_=lv, func=Exp, scale=-0.5)
        nc.vector.tensor_scalar(
            out=x, in0=x, scalar1=mv[:, 0:1], scalar2=rinv,
            op0=sub_op, op1=mul_op,
        )
        nc.vector.tensor_tensor(out=x, in0=x, in1=gmb_sb, op=mul_op)
        nc.vector.tensor_tensor(out=x, in0=x, in1=btb_sb, op=add_op)
        nc.sync.dma_start(out=out_ap[qc], in_=x)

    fin_sb.release()
    at_sb.release()
    at_rbc.release()
    at_ctx.release()
    at_sc.release()
    dram.release()
    persist.release()


_NC_CACHE = {}


def _get_nc():
    if "nc" not in _NC_CACHE:
        _NC_CACHE["nc"] = build_nc()
    return _NC_CACHE["nc"]


def _bf16(x):
    import ml_dtypes
    return np.ascontiguousarray(x.astype(ml_dtypes.bfloat16))


def shard_inputs(inputs):
    import ml_dtypes
    hs = np.ascontiguousarray(np.asarray(inputs["hidden_states"], dtype=np.float32))
    mask = np.asarray(inputs["attention_mask"], dtype=np.float32)
    Wq = np.asarray(inputs["Wq"], dtype=np.float32)
    Wk = np.asarray(inputs["Wk"], dtype=np.float32)
    Wv = np.asarray(inputs["Wv"], dtype=np.float32)
    Wd = np.asarray(inputs["Wd"], dtype=np.float32)
    bq = np.asarray(inputs["bq"], dtype=np.float32)
    bk = np.asarray(inputs["bk"], dtype=np.float32)
    bv = np.asarray(inputs["bv"], dtype=np.float32)
    bd = np.ascontiguousarray(np.asarray(inputs["bd"], dtype=np.float32))
    gam = np.ascontiguousarray(np.asarray(inputs["ln_gamma"], dtype=np.float32))
    bet = np.ascontiguousarray(np.asarray(inputs["ln_beta"], dtype=np.float32))

    hsT = [_bf16(hs[b].T) for b in range(B)]
    mask_b = [np.ascontiguousarray(mask[b, 0, 0, :]) for b in range(B)]
    ones_fr = np.ones((128, 128), np.float32)
    ones_bf = np.ones((128, 64), ml_dtypes.bfloat16)

    in_maps = []
    for c in range(NCORES):
        b, g = c // 4, c % 4
        sl = slice(256 * g, 256 * g + 256)
        # owned token rows: 512*qc + 128*g .. +128 for qc in 0..3
        hs_own = np.stack(
            [hs[b, 512 * qc + 128 * g: 512 * qc + 128 * g + 128] for qc in range(4)]
        )
        in_maps.append({
            "hsT": hsT[b],
            "hs_own": np.ascontiguousarray(hs_own),
            "wqT": _bf16(Wq[sl, :].T),
            "wkT": _bf16(Wk[sl, :].T),
            "wvT": _bf16(Wv[sl, :].T),
            "wdT": _bf16(Wd[:, sl].T),
            "bq_s": np.ascontiguousarray(bq[sl]),
            "bk_s": np.ascontiguousarray(bk[sl]),
            "bv_s": np.ascontiguousarray(bv[sl]),
            "bd_f": bd,
            "gamma_f": gam,
            "beta_f": bet,
            "mask_b": mask_b[b],
            "ones_fr": ones_fr,
            "ones_bf": ones_bf,
        })
    return in_maps


def assemble(results):
    out = np.zeros((B, S, H), np.float32)
    for c in range(NCORES):
        b, g = c // 4, c % 4
        for qc in range(4):
            r0 = 512 * qc + 128 * g
            out[b, r0:r0 + 128, :] = results[c]["out_chunk"][qc]
    return out


LAST_RESULT = None


def kernel(**inputs):
    global LAST_RESULT
    from concourse.bass_utils import run_bass_kernel_spmd

    nc = _get_nc()
    in_maps = shard_inputs(inputs)
    trace = bool(int(os.environ.get("KERNEL_TRACE", "0")))
    res = run_bass_kernel_spmd(nc, in_maps, list(range(NCORES)), trace=trace)
    LAST_RESULT = res
    return assemble(res.results)


def simulate(inputs):
    """CoreSim-based check (no hardware)."""
    from concourse.bass_interp import MultiCoreSim

    nc = _get_nc()
    in_maps = shard_inputs(inputs)
    sim = MultiCoreSim(nc, NCORES)
    for c in range(NCORES):
        for k, v in in_maps[c].items():
            sim.cores[c].tensor(k)[:] = v
    sim.simulate(check_with_hw=False)
    results = [{"out_chunk": np.array(sim.cores[c].tensor("out_chunk"))}
               for c in range(NCORES)]
    return assemble(results)


# revision 19
# speedup vs baseline: 1.5680x; 1.0711x over previous
"""Trainium2 Bass kernel for ALBERT attention (B=2, S=2048, H=1024, NH=16).

Sharding over 8 NeuronCores: 2 batches x 4 head-groups (tensor parallel over
heads within each batch).  Core c handles batch b = c//4 and heads
[4g, 4g+4) where g = c%4.  The kernel pipelines over four 512-token chunks:
for each chunk it runs attention (both head pairs), the partial output
projection, and a ReduceScatter(add) over the batch's 4-core group that both
sums the head-group partials and scatters token ownership; the RS of chunk i
overlaps the attention of chunk i+1.  Core (b, g) ends up owning token rows
512*qc + 128*g .. +128 for qc in 0..3, applies residual + bias + LayerNorm,
and writes those four [128, 1024] slices.

Matmuls run in bf16 (inputs host-cast; fp32 PSUM accumulation).  The softmax
normalization (1/l) and the tiny K=1 broadcast matmuls stay float32r.

Per-core dataflow:
  hsT [1024, 2048] bf16   (host-pretransposed hidden states of its batch)
  qT/kT = W.T-slices @ hsT          (feature-major, head pairs stacked 64+64)
  v     = hsT.T @ WvT-slice         (token-major) + ones column per head
  per 512-token q chunk, per head pair, per 128-key tile:
      scoresT[key, q] = k @ qT   (two heads row-packed, K=64)
      expT = exp(0.125*scoresT + mask[key])   (ScalarE, mask as bias)
      ctx~T[65, q] += [v | 1].T @ expT        (PSUM-accumulated over keys)
  ctxT_h = ctx~T[:64] / ctx~T[64]   (merged reciprocal + K=1 bcast matmul)
  partial(qc) = ctxT(qc).T @ WdT-slice -> ReduceScatter(qc) over group of 4
  out = LN(rs + hs_own + bd) * gamma + beta
"""

import os
import sys

import numpy as np

for _p in ("/opt/trn_rl_repo",):
    if _p not in sys.path:
        sys.path.insert(0, _p)

import concourse.bass as bass
import concourse.mybir as mybir
import concourse.tile as tile
from concourse import bacc
from concourse.bass import ts

F32 = mybir.dt.float32
F32R = mybir.dt.float32r
BF16 = mybir.dt.bfloat16

H, NH, HD = 1024, 16, 64
B, S = 2, 2048
NCORES = 8
GROUPS = [[0, 1, 2, 3], [4, 5, 6, 7]]
CHUNK = 512          # tokens per pipelined chunk (and per-core output rows)
EPS = 1e-12


def build_nc():
    nc = bacc.Bacc(
        "TRN2",
        target_bir_lowering=False,
        debug=False,
        num_devices=NCORES,
    )

    hsT_d = nc.dram_tensor("hsT", [H, S], BF16, kind="ExternalInput")
    hso_d = nc.dram_tensor("hs_own", [4, 128, H], F32, kind="ExternalInput")
    wq_d = nc.dram_tensor("wqT", [H, 256], BF16, kind="ExternalInput")
    wk_d = nc.dram_tensor("wkT", [H, 256], BF16, kind="ExternalInput")
    wv_d = nc.dram_tensor("wvT", [H, 256], BF16, kind="ExternalInput")
    wd_d = nc.dram_tensor("wdT", [256, H], BF16, kind="ExternalInput")
    bq_d = nc.dram_tensor("bq_s", [256], F32, kind="ExternalInput")
    bk_d = nc.dram_tensor("bk_s", [256], F32, kind="ExternalInput")
    bv_d = nc.dram_tensor("bv_s", [256], F32R, kind="ExternalInput")
    bd_d = nc.dram_tensor("bd_f", [H], F32R, kind="ExternalInput")
    gam_d = nc.dram_tensor("gamma_f", [H], F32R, kind="ExternalInput")
    bet_d = nc.dram_tensor("beta_f", [H], F32R, kind="ExternalInput")
    msk_d = nc.dram_tensor("mask_b", [S], F32, kind="ExternalInput")
    onr_d = nc.dram_tensor("ones_fr", [128, 128], F32R, kind="ExternalInput")
    onb_d = nc.dram_tensor("ones_bf", [128, 64], BF16, kind="ExternalInput")
    out_d = nc.dram_tensor("out_chunk", [4, 128, H], F32, kind="ExternalOutput")

    with tile.TileContext(nc) as tc:
        _body(tc, hsT_d, hso_d, wq_d, wk_d, wv_d, wd_d, bq_d, bk_d, bv_d,
              bd_d, gam_d, bet_d, msk_d, onr_d, onb_d, out_d)
    nc.compile()
    return nc


def _body(tc, hsT_d, hso_d, wq_d, wk_d, wv_d, wd_d, bq_d, bk_d, bv_d,
          bd_d, gam_d, bet_d, msk_d, onr_d, onb_d, out_d):
    nc = tc.nc
    Exp = mybir.ActivationFunctionType.Exp
    Sqrt = mybir.ActivationFunctionType.Sqrt
    add_op = mybir.AluOpType.add
    sub_op = mybir.AluOpType.subtract
    mul_op = mybir.AluOpType.mult

    # ---------------- persistent SBUF ----------------
    persist = tc.alloc_tile_pool(name="persist", bufs=1)
    qT_sb = persist.tile([128, 2, S], BF16)      # [dim-in-pair, pair, tok]
    kT_sb = persist.tile([128, 2, S], BF16)
    vaug_sb = persist.tile([128, 16, 4, 65], BF16)  # [key-in-tile, keytile, head, 64v+1]
    ctxT_sb = persist.tile([128, 2, S], BF16)    # normalized ctx, feature-major
    cxo_sb = persist.tile([64, 2, S], BF16)      # odd heads before partition shift
    wd_sb = persist.tile([128, 2, H], BF16)
    bq_sb = persist.tile([128, 2], F32)
    bk_sb = persist.tile([128, 2], F32)
    msk_sb = persist.tile([128, 16], F32)
    onr_sb = persist.tile([128, 128], F32R)
    bv_sb = persist.tile([1, 256], F32R)
    bd_sb = persist.tile([1, H], F32R)
    gam_sb = persist.tile([1, H], F32R)
    bet_sb = persist.tile([1, H], F32R)
    eps_sb = persist.tile([128, 1], F32)

    nc.vector.memset(eps_sb, EPS)
    # ---------------- load pool (released after QKV) ----------------
    load = tc.alloc_tile_pool(name="load", bufs=1)
    hsT_sb = load.tile([128, 8, S], BF16)        # [feat-in-chunk, featchunk, tok]
    wq_sb = load.tile([128, 8, 256], BF16)
    wk_sb = load.tile([128, 8, 256], BF16)
    wv_sb = load.tile([128, 8, 256], BF16)

    # issue order favors the first QKV tiles: wq/wk + first token chunk first
    hsT_src = hsT_d.ap().rearrange("(c p) (q w) -> q p c w", p=128, w=512)
    nc.sync.dma_start(out=wq_sb, in_=wq_d.ap().rearrange("(c p) d -> p c d", p=128))
    nc.sync.dma_start(out=wk_sb, in_=wk_d.ap().rearrange("(c p) d -> p c d", p=128))
    nc.sync.dma_start(out=hsT_sb[:, :, ts(0, 512)], in_=hsT_src[0])
    nc.sync.dma_start(out=wv_sb, in_=wv_d.ap().rearrange("(c p) d -> p c d", p=128))
    nc.sync.dma_start(out=bq_sb, in_=bq_d.ap().rearrange("(c p) -> p c", p=128))
    nc.sync.dma_start(out=bk_sb, in_=bk_d.ap().rearrange("(c p) -> p c", p=128))
    nc.sync.dma_start(out=bv_sb, in_=bv_d.ap().unsqueeze(0))
    nc.sync.dma_start(out=onr_sb, in_=onr_d.ap())
    for t4 in range(1, 4):
        nc.sync.dma_start(out=hsT_sb[:, :, ts(t4, 512)], in_=hsT_src[t4])
    nc.sync.dma_start(
        out=vaug_sb[:, :, :, 64:65],
        in_=onb_d.ap().rearrange("p (a b) -> p a b", a=16).unsqueeze(3),
    )
    nc.sync.dma_start(out=msk_sb, in_=msk_d.ap().rearrange("(t p) -> p t", p=128))
    nc.sync.dma_start(out=wd_sb, in_=wd_d.ap().rearrange("(c p) d -> p c d", p=128))
    nc.sync.dma_start(out=bd_sb, in_=bd_d.ap().unsqueeze(0))
    nc.sync.dma_start(out=gam_sb, in_=gam_d.ap().unsqueeze(0))
    nc.sync.dma_start(out=bet_sb, in_=bet_d.ap().unsqueeze(0))

    # ---------------- QKV projections ----------------
    qkv_ps = tc.alloc_tile_pool(name="qkv_ps", bufs=3, space="PSUM")

    for t4 in range(4):
        for pr in range(2):
            for w_sb, b_sb, o_sb in (
                (wq_sb, bq_sb, qT_sb),
                (wk_sb, bk_sb, kT_sb),
            ):
                ps = qkv_ps.tile([128, 512], F32, tag="qk_ps")
                for kc in range(8):
                    nc.tensor.matmul(
                        ps,
                        lhsT=w_sb[:, kc, ts(pr, 128)],
                        rhs=hsT_sb[:, kc, ts(t4, 512)],
                        start=(kc == 0),
                        stop=(kc == 7),
                    )
                nc.vector.tensor_scalar_add(
                    out=o_sb[:, pr, ts(t4, 512)], in0=ps,
                    scalar1=b_sb[:, pr:pr + 1],
                )
        for tt in range(4):          # token tiles of 128 within this 512 chunk
            t16 = 4 * t4 + tt
            ps = qkv_ps.tile([128, 256], F32, tag="v_ps")
            for kc in range(8):
                nc.tensor.matmul(
                    ps,
                    lhsT=hsT_sb[:, kc, ts(t16, 128)],
                    rhs=wv_sb[:, kc, :],
                    start=(kc == 0),
                    stop=False,
                )
            nc.tensor.matmul(        # + bv broadcast over tokens (K=1, f32r)
                ps,
                lhsT=onr_sb[0:1, 0:128],
                rhs=bv_sb[0:1, :],
                start=False,
                stop=True,
            )
            nc.vector.tensor_copy(
                out=vaug_sb[:, t16, :, 0:64],
                in_=ps.rearrange("p (h d) -> p h d", h=4),
            )

    qkv_ps.release()
    load.release()

    # ------- attention -> dense -> chunked ReduceScatter -> LayerNorm -------
    dram = tc.alloc_tile_pool(name="dram", bufs=1, space="DRAM")
    rs_in = dram.tile([4, CHUNK, H], F32)        # per-qc partial projections
    rs_out = dram.tile([4, 128, H], F32)         # per-qc owned token rows

    at_sc = tc.alloc_tile_pool(name="at_sc", bufs=2, space="PSUM")    # 4 banks
    at_ctx = tc.alloc_tile_pool(name="at_ctx", bufs=1, space="PSUM")  # 2 banks
    at_rbc = tc.alloc_tile_pool(name="at_rbc", bufs=2, space="PSUM")  # 2 banks
    at_sb = tc.alloc_tile_pool(name="at_sb", bufs=3)
    fin_sb = tc.alloc_tile_pool(name="fin_sb", bufs=2)

    hso_sb = fin_sb.tile([128, 4, H], F32, bufs=1)
    nc.sync.dma_start(out=hso_sb, in_=hso_d.ap().rearrange("q p d -> p q d"))

    # broadcast bd / gamma / beta across partitions via K=1 matmuls (f32r)
    bdb_sb = fin_sb.tile([128, H], F32, bufs=1)
    gmb_sb = fin_sb.tile([128, H], F32, bufs=1)
    btb_sb = fin_sb.tile([128, H], F32, bufs=1)
    for src, dst in ((bd_sb, bdb_sb), (gam_sb, gmb_sb), (bet_sb, btb_sb)):
        for nh in range(2):
            pb = at_rbc.tile([128, 512], F32, tag="rbc")
            nc.tensor.matmul(
                pb,
                lhsT=onr_sb[0:1, 0:128],
                rhs=src[0:1, ts(nh, 512)],
            )
            nc.vector.tensor_copy(out=dst[:, ts(nh, 512)], in_=pb)

    out_ap = out_d.ap()

    def norm_unit(qc, pr, ctx_e, ctx_o):
        """DVE-only epilogue of an attention unit: copy ctx~ out of PSUM
        (freeing the banks for the next unit) and start the 1/l chain.
        Returns (cse, cso, lrecr) plus closures of deferred PE work."""
        cse = at_sb.tile([65, 512], F32, tag="cse", bufs=2)
        cso = at_sb.tile([65, 512], F32, tag="cso", bufs=2)
        nc.vector.tensor_copy(out=cse, in_=ctx_e)
        nc.vector.tensor_copy(out=cso, in_=ctx_o)
        lrec = at_sb.tile([128, 1024], F32, tag="lrec", bufs=2)
        nc.vector.tensor_copy(out=lrec[64:65, 0:512], in_=cse[64:65, :])
        nc.vector.tensor_copy(out=lrec[64:65, 512:1024], in_=cso[64:65, :])
        nc.vector.reciprocal(out=lrec[64:65, :], in_=lrec[64:65, :])
        lrecr = at_sb.tile([128, 1024], F32R, tag="lrecr", bufs=2)
        nc.vector.tensor_copy(out=lrecr[64:65, :], in_=lrec[64:65, :])
        return cse, cso, lrecr

    def post_items(qc, units):
        """Deferred PE work for chunk qc: softmax normalization broadcasts,
        the partition shift, the partial dense, and the ReduceScatter.
        Returned as small closures to interleave into the next unit's
        key-tile loop (the PE has slack under the ACT-bound exp stream)."""
        items = []
        for pr in range(2):
            cse, cso, lrecr = units[pr]

            def fn(pr=pr, cse=cse, cso=cso, lrecr=lrecr):
                for hodd, csrc in ((0, cse), (1, cso)):
                    rbc = at_rbc.tile([128, 512], F32, tag="rbc")
                    nc.tensor.matmul(
                        rbc[0:64, :],
                        lhsT=onr_sb[64:65, 0:64],
                        rhs=lrecr[64:65, ts(hodd, 512)],
                        tile_position=(64, 0),
                    )
                    dst = (ctxT_sb if hodd == 0 else cxo_sb)[0:64, pr, ts(qc, 512)]
                    nc.vector.tensor_tensor(
                        out=dst, in0=csrc[0:64, :], in1=rbc[0:64, :], op=mul_op,
                    )
                # odd-head rows into ctxT partitions 64..127 (only DMA can
                # cross partitions)
                nc.sync.dma_start(
                    out=ctxT_sb[64:128, pr, ts(qc, 512)],
                    in_=cxo_sb[0:64, pr, ts(qc, 512)],
                )
            items.append(fn)
        for tt in range(4):
            pj = fin_sb.tile([128, H], F32, tag="pj", bufs=3)
            for nh in range(2):
                def fn(tt=tt, nh=nh, pj=pj):
                    ps = at_rbc.tile([128, 512], F32, tag="rbc")
                    for pr in range(2):
                        nc.tensor.matmul(
                            ps,
                            lhsT=ctxT_sb[:, pr, ts(4 * qc + tt, 128)],
                            rhs=wd_sb[:, pr, ts(nh, 512)],
                            start=(pr == 0),
                            stop=(pr == 1),
                        )
                    nc.vector.tensor_copy(out=pj[:, ts(nh, 512)], in_=ps)
                    if nh == 1:
                        nc.sync.dma_start(out=rs_in[qc, ts(tt, 128), :], in_=pj)
                items.append(fn)

        def fn_rs():
            nc.gpsimd.collective_compute(
                "ReduceScatter",
                add_op,
                replica_groups=GROUPS,
                ins=[rs_in[qc].flatten()],
                outs=[rs_out[qc].flatten()],
            )
        items.append(fn_rs)
        return items

    def attention_unit(qc, pr, deferred):
        ctx_e = at_ctx.tile([65, 512], F32, tag="ctx_e")
        ctx_o = at_ctx.tile([65, 512], F32, tag="ctx_o")

        # software-pipelined: emit ctx(kt-1) after scores(kt) so the PE's
        # in-order stream never stalls on the exp of the current tile
        def emit_ctx(kt, ex):
            nc.tensor.matmul(
                ctx_e,
                lhsT=vaug_sb[:, kt, 2 * pr, :],
                rhs=ex[:, 0:512],
                start=(kt == 0), stop=(kt == 15),
            )
            nc.tensor.matmul(
                ctx_o,
                lhsT=vaug_sb[:, kt, 2 * pr + 1, :],
                rhs=ex[:, 512:1024],
                start=(kt == 0), stop=(kt == 15),
            )

        prev = None
        for kt in range(16):
            sc = at_sc.tile([128, 1024], F32, tag="sc")
            nc.tensor.matmul(
                sc[:, 0:512],
                lhsT=kT_sb[0:64, pr, ts(kt, 128)],
                rhs=qT_sb[0:64, pr, ts(qc, 512)],
            )
            nc.tensor.matmul(
                sc[:, 512:1024],
                lhsT=kT_sb[64:128, pr, ts(kt, 128)],
                rhs=qT_sb[64:128, pr, ts(qc, 512)],
            )
            ex = at_sb.tile([128, 1024], BF16, tag="ex")
            nc.scalar.activation(
                out=ex, in_=sc[:, :], func=Exp,
                bias=msk_sb[:, kt:kt + 1], scale=0.125,
            )
            if prev is not None:
                emit_ctx(*prev)
            if kt >= 6 and deferred:
                deferred.pop(0)()
            prev = (kt, ex)
        emit_ctx(*prev)
        return norm_unit(qc, pr, ctx_e, ctx_o)

    deferred = []
    for qc in range(4):
        units = []
        units.append(attention_unit(qc, 0, deferred))
        units.append(attention_unit(qc, 1, deferred))
        assert not deferred, f"{len(deferred)} deferred items left at qc={qc}"
        deferred = post_items(qc, units)
    # last chunk's post work has nothing left to hide behind
    for fn in deferred:
        fn()

    # ---- residual + bias + LayerNorm, off the attention/RS critical path.
    # rstd = exp(-0.5*ln(var+eps)) keeps all ScalarE work in the
    # natural_log_exp table set (no reloads between exp batches).
    Ln = mybir.ActivationFunctionType.Ln
    for qc in range(4):
        x = fin_sb.tile([128, H], F32, tag="x")
        nc.sync.dma_start(out=x, in_=rs_out[qc])
        nc.vector.tensor_tensor(out=x, in0=x, in1=hso_sb[:, qc, :], op=add_op)
        nc.vector.tensor_tensor(out=x, in0=x, in1=bdb_sb, op=add_op)
        stats = fin_sb.tile([128, 2, 6], F32, tag="stats")
        for i in range(2):
            nc.vector.bn_stats(out=stats[:, i, :], in_=x[:, ts(i, 512)])
        mv = fin_sb.tile([128, 2], F32, tag="mv")
        nc.vector.bn_aggr(out=mv, in_=stats)
        lv = fin_sb.tile([128, 1], F32, tag="lv")
        nc.scalar.activation(out=lv, in_=mv[:, 1:2], func=Ln, bias=eps_sb, scale=1.0)
        rinv = fin_sb.tile([128, 1], F32, tag="rinv")
        nc.scalar.activation(out=rinv, in_=lv, func=Exp, scale=-0.5)
        nc.vector.tensor_scalar(
            out=x, in0=x, scalar1=mv[:, 0:1], scalar2=rinv,
            op0=sub_op, op1=mul_op,
        )
        nc.vector.tensor_tensor(out=x, in0=x, in1=gmb_sb, op=mul_op)
        nc.vector.tensor_tensor(out=x, in0=x, in1=btb_sb, op=add_op)
        nc.sync.dma_start(out=out_ap[qc], in_=x)

    fin_sb.release()
    at_sb.release()
    at_rbc.release()
    at_ctx.release()
    at_sc.release()
    dram.release()
    persist.release()


_NC_CACHE = {}


def _get_nc():
    if "nc" not in _NC_CACHE:
        _NC_CACHE["nc"] = build_nc()
    return _NC_CACHE["nc"]


def _bf16(x):
    import ml_dtypes
    return np.ascontiguousarray(x.astype(ml_dtypes.bfloat16))


def shard_inputs(inputs):
    import ml_dtypes
    hs = np.ascontiguousarray(np.asarray(inputs["hidden_states"], dtype=np.float32))
    mask = np.asarray(inputs["attention_mask"], dtype=np.float32)
    Wq = np.asarray(inputs["Wq"], dtype=np.float32)
    Wk = np.asarray(inputs["Wk"], dtype=np.float32)
    Wv = np.asarray(inputs["Wv"], dtype=np.float32)
    Wd = np.asarray(inputs["Wd"], dtype=np.float32)
    bq = np.asarray(inputs["bq"], dtype=np.float32)
    bk = np.asarray(inputs["bk"], dtype=np.float32)
    bv = np.asarray(inputs["bv"], dtype=np.float32)
    bd = np.ascontiguousarray(np.asarray(inputs["bd"], dtype=np.float32))
    gam = np.ascontiguousarray(np.asarray(inputs["ln_gamma"], dtype=np.float32))
    bet = np.ascontiguousarray(np.asarray(inputs["ln_beta"], dtype=np.float32))

    hsT = [_bf16(hs[b].T) for b in range(B)]
    mask_b = [np.ascontiguousarray(mask[b, 0, 0, :]) for b in range(B)]
    ones_fr = np.ones((128, 128), np.float32)
    ones_bf = np.ones((128, 64), ml_dtypes.bfloat16)

    in_maps = []
    for c in range(NCORES):
        b, g = c // 4, c % 4
        sl = slice(256 * g, 256 * g + 256)
        # owned token rows: 512*qc + 128*g .. +128 for qc in 0..3
        hs_own = np.stack(
            [hs[b, 512 * qc + 128 * g: 512 * qc + 128 * g + 128] for qc in range(4)]
        )
        in_maps.append({
            "hsT": hsT[b],
            "hs_own": np.ascontiguousarray(hs_own),
            "wqT": _bf16(Wq[sl, :].T),
            "wkT": _bf16(Wk[sl, :].T),
            "wvT": _bf16(Wv[sl, :].T),
            "wdT": _bf16(Wd[:, sl].T),
            "bq_s": np.ascontiguousarray(bq[sl]),
            "bk_s": np.ascontiguousarray(bk[sl]),
            "bv_s": np.ascontiguousarray(bv[sl]),
            "bd_f": bd,
            "gamma_f": gam,
            "beta_f": bet,
            "mask_b": mask_b[b],
            "ones_fr": ones_fr,
            "ones_bf": ones_bf,
        })
    return in_maps


def assemble(results):
    out = np.zeros((B, S, H), np.float32)
    for c in range(NCORES):
        b, g = c // 4, c % 4
        for qc in range(4):
            r0 = 512 * qc + 128 * g
            out[b, r0:r0 + 128, :] = results[c]["out_chunk"][qc]
    return out


LAST_RESULT = None


def kernel(**inputs):
    global LAST_RESULT
    from concourse.bass_utils import run_bass_kernel_spmd

    nc = _get_nc()
    in_maps = shard_inputs(inputs)
    trace = bool(int(os.environ.get("KERNEL_TRACE", "0")))
    res = run_bass_kernel_spmd(nc, in_maps, list(range(NCORES)), trace=trace)
    LAST_RESULT = res
    return assemble(res.results)


def simulate(inputs):
    """CoreSim-based check (no hardware)."""
    from concourse.bass_interp import MultiCoreSim

    nc = _get_nc()
    in_maps = shard_inputs(inputs)
    sim = MultiCoreSim(nc, NCORES)
    for c in range(NCORES):
        for k, v in in_maps[c].items():
            sim.cores[c].tensor(k)[:] = v
    sim.simulate(check_with_hw=False)
    results = [{"out_chunk": np.array(sim.cores[c].tensor("out_chunk"))}
               for c in range(NCORES)]
    return assemble(results)


# revision 21
# speedup vs baseline: 1.5773x; 1.0059x over previous
"""Trainium2 Bass kernel for ALBERT attention (B=2, S=2048, H=1024, NH=16).

Sharding over 8 NeuronCores: 2 batches x 4 head-groups (tensor parallel over
heads within each batch).  Core c handles batch b = c//4 and heads
[4g, 4g+4) where g = c%4.  The kernel pipelines over four 512-token chunks:
for each chunk it runs attention (both head pairs), the partial output
projection, and a ReduceScatter(add) over the batch's 4-core group that both
sums the head-group partials and scatters token ownership; the RS of chunk i
overlaps the attention of chunk i+1.  Core (b, g) ends up owning token rows
512*qc + 128*g .. +128 for qc in 0..3, applies residual + bias + LayerNorm,
and writes those four [128, 1024] slices.

Matmuls run in bf16 (inputs host-cast; fp32 PSUM accumulation).  The softmax
normalization (1/l) and the tiny K=1 broadcast matmuls stay float32r.

Per-core dataflow:
  hsT [1024, 2048] bf16   (host-pretransposed hidden states of its batch)
  qT/kT = W.T-slices @ hsT          (feature-major, head pairs stacked 64+64)
  v     = hsT.T @ WvT-slice         (token-major) + ones column per head
  per 512-token q chunk, per head pair, per 128-key tile:
      scoresT[key, q] = k @ qT   (two heads row-packed, K=64)
      expT = exp(0.125*scoresT + mask[key])   (ScalarE, mask as bias)
      ctx~T[65, q] += [v | 1].T @ expT        (PSUM-accumulated over keys)
  ctxT_h = ctx~T[:64] / ctx~T[64]   (merged reciprocal + K=1 bcast matmul)
  partial(qc) = ctxT(qc).T @ WdT-slice -> ReduceScatter(qc) over group of 4
  out = LN(rs + hs_own + bd) * gamma + beta
"""

import os
import sys

import numpy as np

for _p in ("/opt/trn_rl_repo",):
    if _p not in sys.path:
        sys.path.insert(0, _p)

import concourse.bass as bass
import concourse.mybir as mybir
import concourse.tile as tile
from concourse import bacc
from concourse.bass import ts

F32 = mybir.dt.float32
F32R = mybir.dt.float32r
BF16 = mybir.dt.bfloat16

H, NH, HD = 1024, 16, 64
B, S = 2, 2048
NCORES = 8
GROUPS = [[0, 1, 2, 3], [4, 5, 6, 7]]
CHUNK = 512          # tokens per pipelined chunk (and per-core output rows)
EPS = 1e-12


def build_nc():
    nc = bacc.Bacc(
        "TRN2",
        target_bir_lowering=False,
        debug=False,
        num_devices=NCORES,
    )

    hsT_d = nc.dram_tensor("hsT", [H, S], BF16, kind="ExternalInput")
    hso_d = nc.dram_tensor("hs_own", [4, 128, H], F32, kind="ExternalInput")
    wq_d = nc.dram_tensor("wqT", [H, 256], BF16, kind="ExternalInput")
    wk_d = nc.dram_tensor("wkT", [H, 256], BF16, kind="ExternalInput")
    wv_d = nc.dram_tensor("wvT", [H, 256], BF16, kind="ExternalInput")
    wd_d = nc.dram_tensor("wdT", [256, H], BF16, kind="ExternalInput")
    bq_d = nc.dram_tensor("bq_s", [256], F32, kind="ExternalInput")
    bk_d = nc.dram_tensor("bk_s", [256], F32, kind="ExternalInput")
    bv_d = nc.dram_tensor("bv_s", [256], F32R, kind="ExternalInput")
    bd_d = nc.dram_tensor("bd_f", [H], F32R, kind="ExternalInput")
    gam_d = nc.dram_tensor("gamma_f", [H], F32R, kind="ExternalInput")
    bet_d = nc.dram_tensor("beta_f", [H], F32R, kind="ExternalInput")
    msk_d = nc.dram_tensor("mask_b", [S], F32, kind="ExternalInput")
    onr_d = nc.dram_tensor("ones_fr", [128, 128], F32R, kind="ExternalInput")
    onb_d = nc.dram_tensor("ones_bf", [128, 64], BF16, kind="ExternalInput")
    out_d = nc.dram_tensor("out_chunk", [4, 128, H], F32, kind="ExternalOutput")

    with tile.TileContext(nc) as tc:
        _body(tc, hsT_d, hso_d, wq_d, wk_d, wv_d, wd_d, bq_d, bk_d, bv_d,
              bd_d, gam_d, bet_d, msk_d, onr_d, onb_d, out_d)
    nc.compile()
    return nc


def _body(tc, hsT_d, hso_d, wq_d, wk_d, wv_d, wd_d, bq_d, bk_d, bv_d,
          bd_d, gam_d, bet_d, msk_d, onr_d, onb_d, out_d):
    nc = tc.nc
    Exp = mybir.ActivationFunctionType.Exp
    Ln = mybir.ActivationFunctionType.Ln
    add_op = mybir.AluOpType.add
    sub_op = mybir.AluOpType.subtract
    mul_op = mybir.AluOpType.mult

    # ---------------- persistent SBUF ----------------
    persist = tc.alloc_tile_pool(name="persist", bufs=1)
    qT_sb = persist.tile([128, 2, S], BF16)      # [dim-in-pair, pair, tok]
    kT_sb = persist.tile([128, 2, S], BF16)
    vaug_sb = persist.tile([128, 16, 4, 65], BF16)  # [key-in-tile, keytile, head, 64v+1]
    ctxT_sb = persist.tile([128, 2, S], BF16)    # normalized ctx, feature-major
    cxo_sb = persist.tile([64, 2, S], BF16)      # odd heads before partition shift
    wd_sb = persist.tile([128, 2, H], BF16)
    bq_sb = persist.tile([128, 2], F32)
    bk_sb = persist.tile([128, 2], F32)
    msk_sb = persist.tile([128, 16], F32)
    onr_sb = persist.tile([128, 128], F32R)
    bv_sb = persist.tile([1, 256], F32R)
    bd_sb = persist.tile([1, H], F32R)
    gam_sb = persist.tile([1, H], F32R)
    bet_sb = persist.tile([1, H], F32R)
    eps_sb = persist.tile([128, 1], F32)

    nc.vector.memset(eps_sb, EPS)
    # ---------------- load pool (released after QKV) ----------------
    load = tc.alloc_tile_pool(name="load", bufs=1)
    hsT_sb = load.tile([128, 8, S], BF16)        # [feat-in-chunk, featchunk, tok]
    wq_sb = load.tile([128, 8, 256], BF16)
    wk_sb = load.tile([128, 8, 256], BF16)
    wv_sb = load.tile([128, 8, 256], BF16)

    # issue order favors the first QKV tiles: wq/wk + first token chunk first
    hsT_src = hsT_d.ap().rearrange("(c p) (q w) -> q p c w", p=128, w=512)
    nc.sync.dma_start(out=wq_sb, in_=wq_d.ap().rearrange("(c p) d -> p c d", p=128))
    nc.sync.dma_start(out=wk_sb, in_=wk_d.ap().rearrange("(c p) d -> p c d", p=128))
    nc.sync.dma_start(out=hsT_sb[:, :, ts(0, 512)], in_=hsT_src[0])
    nc.sync.dma_start(out=wv_sb, in_=wv_d.ap().rearrange("(c p) d -> p c d", p=128))
    nc.sync.dma_start(out=bq_sb, in_=bq_d.ap().rearrange("(c p) -> p c", p=128))
    nc.sync.dma_start(out=bk_sb, in_=bk_d.ap().rearrange("(c p) -> p c", p=128))
    nc.sync.dma_start(out=bv_sb, in_=bv_d.ap().unsqueeze(0))
    nc.sync.dma_start(out=onr_sb, in_=onr_d.ap())
    for t4 in range(1, 4):
        nc.sync.dma_start(out=hsT_sb[:, :, ts(t4, 512)], in_=hsT_src[t4])
    nc.sync.dma_start(
        out=vaug_sb[:, :, :, 64:65],
        in_=onb_d.ap().rearrange("p (a b) -> p a b", a=16).unsqueeze(3),
    )
    nc.sync.dma_start(out=msk_sb, in_=msk_d.ap().rearrange("(t p) -> p t", p=128))
    nc.sync.dma_start(out=wd_sb, in_=wd_d.ap().rearrange("(c p) d -> p c d", p=128))
    nc.sync.dma_start(out=bd_sb, in_=bd_d.ap().unsqueeze(0))
    nc.sync.dma_start(out=gam_sb, in_=gam_d.ap().unsqueeze(0))
    nc.sync.dma_start(out=bet_sb, in_=bet_d.ap().unsqueeze(0))

    # ---------------- QKV projections ----------------
    qkv_ps = tc.alloc_tile_pool(name="qkv_ps", bufs=3, space="PSUM")

    for t4 in range(4):
        for pr in range(2):
            # interleave the q and k accumulation chains so consecutive
            # matmuls hit different PSUM banks (drains overlap fills)
            psq = qkv_ps.tile([128, 512], F32, tag="qk_ps")
            psk = qkv_ps.tile([128, 512], F32, tag="qk_ps")
            for kc in range(8):
                nc.tensor.matmul(
                    psq,
                    lhsT=wq_sb[:, kc, ts(pr, 128)],
                    rhs=hsT_sb[:, kc, ts(t4, 512)],
                    start=(kc == 0),
                    stop=(kc == 7),
                )
                nc.tensor.matmul(
                    psk,
                    lhsT=wk_sb[:, kc, ts(pr, 128)],
                    rhs=hsT_sb[:, kc, ts(t4, 512)],
                    start=(kc == 0),
                    stop=(kc == 7),
                )
            nc.vector.tensor_scalar_add(
                out=qT_sb[:, pr, ts(t4, 512)], in0=psq,
                scalar1=bq_sb[:, pr:pr + 1],
            )
            nc.vector.tensor_scalar_add(
                out=kT_sb[:, pr, ts(t4, 512)], in0=psk,
                scalar1=bk_sb[:, pr:pr + 1],
            )
        for tp in range(2):          # pairs of 128-token tiles, interleaved
            ta, tb = 4 * t4 + 2 * tp, 4 * t4 + 2 * tp + 1
            psa = qkv_ps.tile([128, 256], F32, tag="v_ps")
            psb = qkv_ps.tile([128, 256], F32, tag="v_ps")
            for kc in range(8):
                nc.tensor.matmul(
                    psa,
                    lhsT=hsT_sb[:, kc, ts(ta, 128)],
                    rhs=wv_sb[:, kc, :],
                    start=(kc == 0),
                    stop=False,
                )
                nc.tensor.matmul(
                    psb,
                    lhsT=hsT_sb[:, kc, ts(tb, 128)],
                    rhs=wv_sb[:, kc, :],
                    start=(kc == 0),
                    stop=False,
                )
            for t16, ps in ((ta, psa), (tb, psb)):
                nc.tensor.matmul(    # + bv broadcast over tokens (K=1, f32r)
                    ps,
                    lhsT=onr_sb[0:1, 0:128],
                    rhs=bv_sb[0:1, :],
                    start=False,
                    stop=True,
                )
                nc.vector.tensor_copy(
                    out=vaug_sb[:, t16, :, 0:64],
                    in_=ps.rearrange("p (h d) -> p h d", h=4),
                )

    qkv_ps.release()
    load.release()

    # ------- attention -> dense -> chunked ReduceScatter -> LayerNorm -------
    dram = tc.alloc_tile_pool(name="dram", bufs=1, space="DRAM")
    rs_in = dram.tile([4, CHUNK, H], F32)        # per-qc partial projections
    rs_out = dram.tile([4, 128, H], F32)         # per-qc owned token rows

    at_sc = tc.alloc_tile_pool(name="at_sc", bufs=2, space="PSUM")    # 4 banks
    at_ctx = tc.alloc_tile_pool(name="at_ctx", bufs=1, space="PSUM")  # 2 banks
    at_rbc = tc.alloc_tile_pool(name="at_rbc", bufs=2, space="PSUM")  # 2 banks
    at_sb = tc.alloc_tile_pool(name="at_sb", bufs=3)
    fin_sb = tc.alloc_tile_pool(name="fin_sb", bufs=2)

    hso_sb = fin_sb.tile([128, 4, H], F32, bufs=1)
    nc.sync.dma_start(out=hso_sb, in_=hso_d.ap().rearrange("q p d -> p q d"))

    # broadcast bd / gamma / beta across partitions via K=1 matmuls (f32r)
    bdb_sb = fin_sb.tile([128, H], F32, bufs=1)
    gmb_sb = fin_sb.tile([128, H], F32, bufs=1)
    btb_sb = fin_sb.tile([128, H], F32, bufs=1)
    for src, dst in ((bd_sb, bdb_sb), (gam_sb, gmb_sb), (bet_sb, btb_sb)):
        for nh in range(2):
            pb = at_rbc.tile([128, 512], F32, tag="rbc")
            nc.tensor.matmul(
                pb,
                lhsT=onr_sb[0:1, 0:128],
                rhs=src[0:1, ts(nh, 512)],
            )
            nc.vector.tensor_copy(out=dst[:, ts(nh, 512)], in_=pb)

    out_ap = out_d.ap()

    def norm_unit(qc, pr, ctx_e, ctx_o):
        """DVE-only epilogue of an attention unit: copy ctx~ out of PSUM
        (freeing the banks for the next unit) and start the 1/l chain.
        Returns (cse, cso, lrecr) plus closures of deferred PE work."""
        cse = at_sb.tile([65, 512], F32, tag="cse", bufs=2)
        cso = at_sb.tile([65, 512], F32, tag="cso", bufs=2)
        nc.vector.tensor_copy(out=cse, in_=ctx_e)
        nc.vector.tensor_copy(out=cso, in_=ctx_o)
        lrec = at_sb.tile([128, 1024], F32, tag="lrec", bufs=2)
        nc.vector.tensor_copy(out=lrec[64:65, 0:512], in_=cse[64:65, :])
        nc.vector.tensor_copy(out=lrec[64:65, 512:1024], in_=cso[64:65, :])
        # 1/l = exp(-ln(l)) on ScalarE: keeps the slow single-lane reciprocal
        # out of the DVE FIFO (deferred muls queue behind it otherwise), and
        # Ln/Exp share one ACT table set with the attention exps.
        lnl = at_sb.tile([128, 1024], F32, tag="lnl", bufs=2)
        nc.scalar.activation(out=lnl[64:65, :], in_=lrec[64:65, :], func=Ln)
        lrecr = at_sb.tile([128, 1024], F32R, tag="lrecr", bufs=2)
        nc.scalar.activation(out=lrecr[64:65, :], in_=lnl[64:65, :], func=Exp,
                             scale=-1.0)
        return cse, cso, lrecr

    def post_items(qc, units):
        """Deferred PE work for chunk qc: softmax normalization broadcasts,
        the partition shift, the partial dense, and the ReduceScatter.
        Returned as small closures to interleave into the next unit's
        key-tile loop (the PE has slack under the ACT-bound exp stream)."""
        items = []
        for pr in range(2):
            cse, cso, lrecr = units[pr]

            def fn(pr=pr, cse=cse, cso=cso, lrecr=lrecr):
                for hodd, csrc in ((0, cse), (1, cso)):
                    rbc = at_rbc.tile([128, 512], F32, tag="rbc")
                    nc.tensor.matmul(
                        rbc[0:64, :],
                        lhsT=onr_sb[64:65, 0:64],
                        rhs=lrecr[64:65, ts(hodd, 512)],
                        tile_position=(64, 0),
                    )
                    dst = (ctxT_sb if hodd == 0 else cxo_sb)[0:64, pr, ts(qc, 512)]
                    nc.vector.tensor_tensor(
                        out=dst, in0=csrc[0:64, :], in1=rbc[0:64, :], op=mul_op,
                    )
                # odd-head rows into ctxT partitions 64..127 (only DMA can
                # cross partitions)
                nc.sync.dma_start(
                    out=ctxT_sb[64:128, pr, ts(qc, 512)],
                    in_=cxo_sb[0:64, pr, ts(qc, 512)],
                )
            items.append(fn)
        for tt in range(4):
            pj = fin_sb.tile([128, H], F32, tag="pj", bufs=3)
            for nh in range(2):
                def fn(tt=tt, nh=nh, pj=pj):
                    ps = at_rbc.tile([128, 512], F32, tag="rbc")
                    for pr in range(2):
                        nc.tensor.matmul(
                            ps,
                            lhsT=ctxT_sb[:, pr, ts(4 * qc + tt, 128)],
                            rhs=wd_sb[:, pr, ts(nh, 512)],
                            start=(pr == 0),
                            stop=(pr == 1),
                        )
                    nc.vector.tensor_copy(out=pj[:, ts(nh, 512)], in_=ps)
                    if nh == 1:
                        nc.sync.dma_start(out=rs_in[qc, ts(tt, 128), :], in_=pj)
                items.append(fn)

        def fn_rs():
            nc.gpsimd.collective_compute(
                "ReduceScatter",
                add_op,
                replica_groups=GROUPS,
                ins=[rs_in[qc].flatten()],
                outs=[rs_out[qc].flatten()],
            )
        items.append(fn_rs)
        return items

    def attention_unit(qc, pr, deferred):
        ctx_e = at_ctx.tile([65, 512], F32, tag="ctx_e")
        ctx_o = at_ctx.tile([65, 512], F32, tag="ctx_o")

        # software-pipelined: emit ctx(kt-1) after scores(kt) so the PE's
        # in-order stream never stalls on the exp of the current tile
        def emit_ctx(kt, ex):
            nc.tensor.matmul(
                ctx_e,
                lhsT=vaug_sb[:, kt, 2 * pr, :],
                rhs=ex[:, 0:512],
                start=(kt == 0), stop=(kt == 15),
            )
            nc.tensor.matmul(
                ctx_o,
                lhsT=vaug_sb[:, kt, 2 * pr + 1, :],
                rhs=ex[:, 512:1024],
                start=(kt == 0), stop=(kt == 15),
            )

        prev = None
        for kt in range(16):
            sc = at_sc.tile([128, 1024], F32, tag="sc")
            nc.tensor.matmul(
                sc[:, 0:512],
                lhsT=kT_sb[0:64, pr, ts(kt, 128)],
                rhs=qT_sb[0:64, pr, ts(qc, 512)],
            )
            nc.tensor.matmul(
                sc[:, 512:1024],
                lhsT=kT_sb[64:128, pr, ts(kt, 128)],
                rhs=qT_sb[64:128, pr, ts(qc, 512)],
            )
            ex = at_sb.tile([128, 1024], BF16, tag="ex")
            nc.scalar.activation(
                out=ex, in_=sc[:, :], func=Exp,
                bias=msk_sb[:, kt:kt + 1], scale=0.125,
            )
            if prev is not None:
                emit_ctx(*prev)
            if kt >= 6 and deferred:
                deferred.pop(0)()
            prev = (kt, ex)
        emit_ctx(*prev)
        return norm_unit(qc, pr, ctx_e, ctx_o)

    deferred = []
    for qc in range(4):
        units = []
        units.append(attention_unit(qc, 0, deferred))
        units.append(attention_unit(qc, 1, deferred))
        assert not deferred, f"{len(deferred)} deferred items left at qc={qc}"
        deferred = post_items(qc, units)
    # last chunk's post work has nothing left to hide behind
    for fn in deferred:
        fn()

    # ---- residual + bias + LayerNorm, off the attention/RS critical path.
    # rstd = exp(-0.5*ln(var+eps)) keeps all ScalarE work in the
    # natural_log_exp table set (no reloads between exp batches).
    for qc in range(4):
        x = fin_sb.tile([128, H], F32, tag="x")
        nc.sync.dma_start(out=x, in_=rs_out[qc])
        nc.vector.tensor_tensor(out=x, in0=x, in1=hso_sb[:, qc, :], op=add_op)
        nc.vector.tensor_tensor(out=x, in0=x, in1=bdb_sb, op=add_op)
        stats = fin_sb.tile([128, 2, 6], F32, tag="stats")
        for i in range(2):
            nc.vector.bn_stats(out=stats[:, i, :], in_=x[:, ts(i, 512)])
        mv = fin_sb.tile([128, 2], F32, tag="mv")
        nc.vector.bn_aggr(out=mv, in_=stats)
        lv = fin_sb.tile([128, 1], F32, tag="lv")
        nc.scalar.activation(out=lv, in_=mv[:, 1:2], func=Ln, bias=eps_sb, scale=1.0)
        rinv = fin_sb.tile([128, 1], F32, tag="rinv")
        nc.scalar.activation(out=rinv, in_=lv, func=Exp, scale=-0.5)
        nc.vector.tensor_scalar(
            out=x, in0=x, scalar1=mv[:, 0:1], scalar2=rinv,
            op0=sub_op, op1=mul_op,
        )
        nc.vector.tensor_tensor(out=x, in0=x, in1=gmb_sb, op=mul_op)
        nc.vector.tensor_tensor(out=x, in0=x, in1=btb_sb, op=add_op)
        nc.sync.dma_start(out=out_ap[qc], in_=x)

    fin_sb.release()
    at_sb.release()
    at_rbc.release()
    at_ctx.release()
    at_sc.release()
    dram.release()
    persist.release()


_NC_CACHE = {}


def _get_nc():
    if "nc" not in _NC_CACHE:
        _NC_CACHE["nc"] = build_nc()
    return _NC_CACHE["nc"]


def _bf16(x):
    import ml_dtypes
    return np.ascontiguousarray(x.astype(ml_dtypes.bfloat16))


def shard_inputs(inputs):
    import ml_dtypes
    hs = np.ascontiguousarray(np.asarray(inputs["hidden_states"], dtype=np.float32))
    mask = np.asarray(inputs["attention_mask"], dtype=np.float32)
    Wq = np.asarray(inputs["Wq"], dtype=np.float32)
    Wk = np.asarray(inputs["Wk"], dtype=np.float32)
    Wv = np.asarray(inputs["Wv"], dtype=np.float32)
    Wd = np.asarray(inputs["Wd"], dtype=np.float32)
    bq = np.asarray(inputs["bq"], dtype=np.float32)
    bk = np.asarray(inputs["bk"], dtype=np.float32)
    bv = np.asarray(inputs["bv"], dtype=np.float32)
    bd = np.ascontiguousarray(np.asarray(inputs["bd"], dtype=np.float32))
    gam = np.ascontiguousarray(np.asarray(inputs["ln_gamma"], dtype=np.float32))
    bet = np.ascontiguousarray(np.asarray(inputs["ln_beta"], dtype=np.float32))

    hsT = [_bf16(hs[b].T) for b in range(B)]
    mask_b = [np.ascontiguousarray(mask[b, 0, 0, :]) for b in range(B)]
    ones_fr = np.ones((128, 128), np.float32)
    ones_bf = np.ones((128, 64), ml_dtypes.bfloat16)

    in_maps = []
    for c in range(NCORES):
        b, g = c // 4, c % 4
        sl = slice(256 * g, 256 * g + 256)
        # owned token rows: 512*qc + 128*g .. +128 for qc in 0..3
        hs_own = np.stack(
            [hs[b, 512 * qc + 128 * g: 512 * qc + 128 * g + 128] for qc in range(4)]
        )
        in_maps.append({
            "hsT": hsT[b],
            "hs_own": np.ascontiguousarray(hs_own),
            "wqT": _bf16(Wq[sl, :].T),
            "wkT": _bf16(Wk[sl, :].T),
            "wvT": _bf16(Wv[sl, :].T),
            "wdT": _bf16(Wd[:, sl].T),
            "bq_s": np.ascontiguousarray(bq[sl]),
            "bk_s": np.ascontiguousarray(bk[sl]),
            "bv_s": np.ascontiguousarray(bv[sl]),
            "bd_f": bd,
            "gamma_f": gam,
            "beta_f": bet,
            "mask_b": mask_b[b],
            "ones_fr": ones_fr,
            "ones_bf": ones_bf,
        })
    return in_maps


def assemble(results):
    out = np.zeros((B, S, H), np.float32)
    for c in range(NCORES):
        b, g = c // 4, c % 4
        for qc in range(4):
            r0 = 512 * qc + 128 * g
            out[b, r0:r0 + 128, :] = results[c]["out_chunk"][qc]
    return out


LAST_RESULT = None


def kernel(**inputs):
    global LAST_RESULT
    from concourse.bass_utils import run_bass_kernel_spmd

    nc = _get_nc()
    in_maps = shard_inputs(inputs)
    trace = bool(int(os.environ.get("KERNEL_TRACE", "0")))
    res = run_bass_kernel_spmd(nc, in_maps, list(range(NCORES)), trace=trace)
    LAST_RESULT = res
    return assemble(res.results)


def simulate(inputs):
    """CoreSim-based check (no hardware)."""
    from concourse.bass_interp import MultiCoreSim

    nc = _get_nc()
    in_maps = shard_inputs(inputs)
    sim = MultiCoreSim(nc, NCORES)
    for c in range(NCORES):
        for k, v in in_maps[c].items():
            sim.cores[c].tensor(k)[:] = v
    sim.simulate(check_with_hw=False)
    results = [{"out_chunk": np.array(sim.cores[c].tensor("out_chunk"))}
               for c in range(NCORES)]
    return assemble(results)


# revision 23
# speedup vs baseline: 1.5773x; 1.0000x over previous
"""Trainium2 Bass kernel for ALBERT attention (B=2, S=2048, H=1024, NH=16).

Sharding over 8 NeuronCores: 2 batches x 4 head-groups (tensor parallel over
heads within each batch).  Core c handles batch b = c//4 and heads
[4g, 4g+4) where g = c%4.  The kernel pipelines over four 512-token chunks:
for each chunk it runs attention (both head pairs), the partial output
projection, and a ReduceScatter(add) over the batch's 4-core group that both
sums the head-group partials and scatters token ownership; the RS of chunk i
overlaps the attention of chunk i+1.  Core (b, g) ends up owning token rows
512*qc + 128*g .. +128 for qc in 0..3, applies residual + bias + LayerNorm,
and writes those four [128, 1024] slices.

Matmuls run in bf16 (inputs host-cast; fp32 PSUM accumulation).  The softmax
normalization (1/l) and the tiny K=1 broadcast matmuls stay float32r.

Per-core dataflow:
  hsT [1024, 2048] bf16   (host-pretransposed hidden states of its batch)
  qT/kT = W.T-slices @ hsT          (feature-major, head pairs stacked 64+64)
  v     = hsT.T @ WvT-slice         (token-major) + ones column per head
  per 512-token q chunk, per head pair, per 128-key tile:
      scoresT[key, q] = k @ qT   (two heads row-packed, K=64)
      expT = exp(0.125*scoresT + mask[key])   (ScalarE, mask as bias)
      ctx~T[65, q] += [v | 1].T @ expT        (PSUM-accumulated over keys)
  ctxT_h = ctx~T[:64] / ctx~T[64]   (merged reciprocal + K=1 bcast matmul)
  partial(qc) = ctxT(qc).T @ WdT-slice -> ReduceScatter(qc) over group of 4
  out = LN(rs + hs_own + bd) * gamma + beta
"""

import os
import sys

import numpy as np

for _p in ("/opt/trn_rl_repo",):
    if _p not in sys.path:
        sys.path.insert(0, _p)

import concourse.bass as bass
import concourse.mybir as mybir
import concourse.tile as tile
from concourse import bacc
from concourse.bass import ts

F32 = mybir.dt.float32
F32R = mybir.dt.float32r
BF16 = mybir.dt.bfloat16

H, NH, HD = 1024, 16, 64
B, S = 2, 2048
NCORES = 8
GROUPS = [[0, 1, 2, 3], [4, 5, 6, 7]]
CHUNK = 512          # tokens per pipelined chunk (and per-core output rows)
EPS = 1e-12


def build_nc():
    nc = bacc.Bacc(
        "TRN2",
        target_bir_lowering=False,
        debug=False,
        num_devices=NCORES,
    )

    hsT_d = nc.dram_tensor("hsT", [H, S], BF16, kind="ExternalInput")
    hso_d = nc.dram_tensor("hs_own", [4, 128, H], F32, kind="ExternalInput")
    wq_d = nc.dram_tensor("wqT", [H, 256], BF16, kind="ExternalInput")
    wk_d = nc.dram_tensor("wkT", [H, 256], BF16, kind="ExternalInput")
    wv_d = nc.dram_tensor("wvT", [H, 256], BF16, kind="ExternalInput")
    wd_d = nc.dram_tensor("wdT", [256, H], BF16, kind="ExternalInput")
    bq_d = nc.dram_tensor("bq_s", [256], F32, kind="ExternalInput")
    bk_d = nc.dram_tensor("bk_s", [256], F32, kind="ExternalInput")
    bv_d = nc.dram_tensor("bv_s", [256], F32R, kind="ExternalInput")
    bd_d = nc.dram_tensor("bd_f", [H], F32R, kind="ExternalInput")
    gam_d = nc.dram_tensor("gamma_f", [H], F32R, kind="ExternalInput")
    bet_d = nc.dram_tensor("beta_f", [H], F32R, kind="ExternalInput")
    msk_d = nc.dram_tensor("mask_b", [S], F32, kind="ExternalInput")
    onr_d = nc.dram_tensor("ones_fr", [128, 128], F32R, kind="ExternalInput")
    onb_d = nc.dram_tensor("ones_bf", [128, 64], BF16, kind="ExternalInput")
    out_d = nc.dram_tensor("out_chunk", [4, 128, H], F32, kind="ExternalOutput")

    with tile.TileContext(nc) as tc:
        _body(tc, hsT_d, hso_d, wq_d, wk_d, wv_d, wd_d, bq_d, bk_d, bv_d,
              bd_d, gam_d, bet_d, msk_d, onr_d, onb_d, out_d)
    nc.compile()
    return nc


def _body(tc, hsT_d, hso_d, wq_d, wk_d, wv_d, wd_d, bq_d, bk_d, bv_d,
          bd_d, gam_d, bet_d, msk_d, onr_d, onb_d, out_d):
    nc = tc.nc
    Exp = mybir.ActivationFunctionType.Exp
    Ln = mybir.ActivationFunctionType.Ln
    add_op = mybir.AluOpType.add
    sub_op = mybir.AluOpType.subtract
    mul_op = mybir.AluOpType.mult

    # ---------------- persistent SBUF ----------------
    persist = tc.alloc_tile_pool(name="persist", bufs=1)
    qT_sb = persist.tile([128, 2, S], BF16)      # [dim-in-pair, pair, tok]
    kT_sb = persist.tile([128, 2, S], BF16)
    vaug_sb = persist.tile([128, 16, 4, 65], BF16)  # [key-in-tile, keytile, head, 64v+1]
    ctxT_sb = persist.tile([128, 2, S], BF16)    # normalized ctx, feature-major
    cxo_sb = persist.tile([64, 2, S], BF16)      # odd heads before partition shift
    wd_sb = persist.tile([128, 2, H], BF16)
    bq_sb = persist.tile([128, 2], F32)
    bk_sb = persist.tile([128, 2], F32)
    msk_sb = persist.tile([128, 16], F32)
    onr_sb = persist.tile([128, 128], F32R)
    bv_sb = persist.tile([1, 256], F32R)
    bd_sb = persist.tile([1, H], F32R)
    gam_sb = persist.tile([1, H], F32R)
    bet_sb = persist.tile([1, H], F32R)
    eps_sb = persist.tile([128, 1], F32)

    nc.vector.memset(eps_sb, EPS)
    # ---------------- load pool (released after QKV) ----------------
    load = tc.alloc_tile_pool(name="load", bufs=1)
    hsT_sb = load.tile([128, 8, S], BF16)        # [feat-in-chunk, featchunk, tok]
    wq_sb = load.tile([128, 8, 256], BF16)
    wk_sb = load.tile([128, 8, 256], BF16)
    wv_sb = load.tile([128, 8, 256], BF16)

    # issue order favors the first QKV tiles: wq/wk + first token chunk first
    hsT_src = hsT_d.ap().rearrange("(c p) (q w) -> q p c w", p=128, w=512)
    nc.sync.dma_start(out=wq_sb, in_=wq_d.ap().rearrange("(c p) d -> p c d", p=128))
    nc.sync.dma_start(out=wk_sb, in_=wk_d.ap().rearrange("(c p) d -> p c d", p=128))
    nc.sync.dma_start(out=hsT_sb[:, :, ts(0, 512)], in_=hsT_src[0])
    nc.sync.dma_start(out=wv_sb, in_=wv_d.ap().rearrange("(c p) d -> p c d", p=128))
    nc.sync.dma_start(out=bq_sb, in_=bq_d.ap().rearrange("(c p) -> p c", p=128))
    nc.sync.dma_start(out=bk_sb, in_=bk_d.ap().rearrange("(c p) -> p c", p=128))
    nc.sync.dma_start(out=bv_sb, in_=bv_d.ap().unsqueeze(0))
    nc.sync.dma_start(out=onr_sb, in_=onr_d.ap())
    for t4 in range(1, 4):
        nc.sync.dma_start(out=hsT_sb[:, :, ts(t4, 512)], in_=hsT_src[t4])
    nc.sync.dma_start(
        out=vaug_sb[:, :, :, 64:65],
        in_=onb_d.ap().rearrange("p (a b) -> p a b", a=16).unsqueeze(3),
    )
    nc.sync.dma_start(out=msk_sb, in_=msk_d.ap().rearrange("(t p) -> p t", p=128))
    nc.sync.dma_start(out=wd_sb, in_=wd_d.ap().rearrange("(c p) d -> p c d", p=128))
    nc.sync.dma_start(out=bd_sb, in_=bd_d.ap().unsqueeze(0))
    nc.sync.dma_start(out=gam_sb, in_=gam_d.ap().unsqueeze(0))
    nc.sync.dma_start(out=bet_sb, in_=bet_d.ap().unsqueeze(0))

    # ---------------- QKV projections ----------------
    qkv_ps = tc.alloc_tile_pool(name="qkv_ps", bufs=3, space="PSUM")

    # kc-outer with 4 live PSUM banks: one LDWEIGHTS per (kc, pr) serves 4
    # matmuls, and consecutive matmuls hit different banks so drains overlap
    for pr in range(2):
        for w_sb, b_sb, o_sb in (
            (wq_sb, bq_sb, qT_sb),
            (wk_sb, bk_sb, kT_sb),
        ):
            pss = [qkv_ps.tile([128, 512], F32, tag="qk_ps", bufs=4,
                                name=f"qk_ps_{i}")
                   for i in range(4)]
            for kc in range(8):
                for t4 in range(4):
                    nc.tensor.matmul(
                        pss[t4],
                        lhsT=w_sb[:, kc, ts(pr, 128)],
                        rhs=hsT_sb[:, kc, ts(t4, 512)],
                        start=(kc == 0),
                        stop=(kc == 7),
                    )
            for t4 in range(4):
                nc.vector.tensor_scalar_add(
                    out=o_sb[:, pr, ts(t4, 512)], in0=pss[t4],
                    scalar1=b_sb[:, pr:pr + 1],
                )
    for t4 in range(4):
        for tp in range(2):          # pairs of 128-token tiles, interleaved
            ta, tb = 4 * t4 + 2 * tp, 4 * t4 + 2 * tp + 1
            psa = qkv_ps.tile([128, 256], F32, tag="v_ps")
            psb = qkv_ps.tile([128, 256], F32, tag="v_ps")
            for kc in range(8):
                nc.tensor.matmul(
                    psa,
                    lhsT=hsT_sb[:, kc, ts(ta, 128)],
                    rhs=wv_sb[:, kc, :],
                    start=(kc == 0),
                    stop=False,
                )
                nc.tensor.matmul(
                    psb,
                    lhsT=hsT_sb[:, kc, ts(tb, 128)],
                    rhs=wv_sb[:, kc, :],
                    start=(kc == 0),
                    stop=False,
                )
            for t16, ps in ((ta, psa), (tb, psb)):
                nc.tensor.matmul(    # + bv broadcast over tokens (K=1, f32r)
                    ps,
                    lhsT=onr_sb[0:1, 0:128],
                    rhs=bv_sb[0:1, :],
                    start=False,
                    stop=True,
                )
                nc.vector.tensor_copy(
                    out=vaug_sb[:, t16, :, 0:64],
                    in_=ps.rearrange("p (h d) -> p h d", h=4),
                )

    qkv_ps.release()
    load.release()

    # ------- attention -> dense -> chunked ReduceScatter -> LayerNorm -------
    dram = tc.alloc_tile_pool(name="dram", bufs=1, space="DRAM")
    rs_in = dram.tile([4, CHUNK, H], F32)        # per-qc partial projections
    rs_out = dram.tile([4, 128, H], F32)         # per-qc owned token rows

    at_sc = tc.alloc_tile_pool(name="at_sc", bufs=2, space="PSUM")    # 4 banks
    at_ctx = tc.alloc_tile_pool(name="at_ctx", bufs=1, space="PSUM")  # 2 banks
    at_rbc = tc.alloc_tile_pool(name="at_rbc", bufs=2, space="PSUM")  # 2 banks
    at_sb = tc.alloc_tile_pool(name="at_sb", bufs=3)
    fin_sb = tc.alloc_tile_pool(name="fin_sb", bufs=2)

    hso_sb = fin_sb.tile([128, 4, H], F32, bufs=1)
    nc.sync.dma_start(out=hso_sb, in_=hso_d.ap().rearrange("q p d -> p q d"))

    # broadcast bd / gamma / beta across partitions via K=1 matmuls (f32r)
    bdb_sb = fin_sb.tile([128, H], F32, bufs=1)
    gmb_sb = fin_sb.tile([128, H], F32, bufs=1)
    btb_sb = fin_sb.tile([128, H], F32, bufs=1)
    for src, dst in ((bd_sb, bdb_sb), (gam_sb, gmb_sb), (bet_sb, btb_sb)):
        for nh in range(2):
            pb = at_rbc.tile([128, 512], F32, tag="rbc")
            nc.tensor.matmul(
                pb,
                lhsT=onr_sb[0:1, 0:128],
                rhs=src[0:1, ts(nh, 512)],
            )
            nc.vector.tensor_copy(out=dst[:, ts(nh, 512)], in_=pb)

    out_ap = out_d.ap()

    def norm_unit(qc, pr, ctx_e, ctx_o):
        """DVE-only epilogue of an attention unit: copy ctx~ out of PSUM
        (freeing the banks for the next unit) and start the 1/l chain.
        Returns (cse, cso, lrecr) plus closures of deferred PE work."""
        cse = at_sb.tile([65, 512], F32, tag="cse", bufs=2)
        cso = at_sb.tile([65, 512], F32, tag="cso", bufs=2)
        nc.vector.tensor_copy(out=cse, in_=ctx_e)
        nc.vector.tensor_copy(out=cso, in_=ctx_o)
        lrec = at_sb.tile([128, 1024], F32, tag="lrec", bufs=2)
        nc.vector.tensor_copy(out=lrec[64:65, 0:512], in_=cse[64:65, :])
        nc.vector.tensor_copy(out=lrec[64:65, 512:1024], in_=cso[64:65, :])
        # 1/l = exp(-ln(l)) on ScalarE: keeps the slow single-lane reciprocal
        # out of the DVE FIFO (deferred muls queue behind it otherwise), and
        # Ln/Exp share one ACT table set with the attention exps.
        lnl = at_sb.tile([128, 1024], F32, tag="lnl", bufs=2)
        nc.scalar.activation(out=lnl[64:65, :], in_=lrec[64:65, :], func=Ln)
        lrecr = at_sb.tile([128, 1024], F32R, tag="lrecr", bufs=2)
        nc.scalar.activation(out=lrecr[64:65, :], in_=lnl[64:65, :], func=Exp,
                             scale=-1.0)
        return cse, cso, lrecr

    def post_items(qc, units):
        """Deferred PE work for chunk qc: softmax normalization broadcasts,
        the partition shift, the partial dense, and the ReduceScatter.
        Returned as small closures to interleave into the next unit's
        key-tile loop (the PE has slack under the ACT-bound exp stream)."""
        items = []
        for pr in range(2):
            cse, cso, lrecr = units[pr]

            def fn(pr=pr, cse=cse, cso=cso, lrecr=lrecr):
                for hodd, csrc in ((0, cse), (1, cso)):
                    rbc = at_rbc.tile([128, 512], F32, tag="rbc")
                    nc.tensor.matmul(
                        rbc[0:64, :],
                        lhsT=onr_sb[64:65, 0:64],
                        rhs=lrecr[64:65, ts(hodd, 512)],
                        tile_position=(64, 0),
                    )
                    dst = (ctxT_sb if hodd == 0 else cxo_sb)[0:64, pr, ts(qc, 512)]
                    nc.vector.tensor_tensor(
                        out=dst, in0=csrc[0:64, :], in1=rbc[0:64, :], op=mul_op,
                    )
                # odd-head rows into ctxT partitions 64..127 (only DMA can
                # cross partitions)
                nc.sync.dma_start(
                    out=ctxT_sb[64:128, pr, ts(qc, 512)],
                    in_=cxo_sb[0:64, pr, ts(qc, 512)],
                )
            items.append(fn)
        for tt in range(4):
            pj = fin_sb.tile([128, H], F32, tag="pj", bufs=3)
            for nh in range(2):
                def fn(tt=tt, nh=nh, pj=pj):
                    ps = at_rbc.tile([128, 512], F32, tag="rbc")
                    for pr in range(2):
                        nc.tensor.matmul(
                            ps,
                            lhsT=ctxT_sb[:, pr, ts(4 * qc + tt, 128)],
                            rhs=wd_sb[:, pr, ts(nh, 512)],
                            start=(pr == 0),
                            stop=(pr == 1),
                        )
                    nc.vector.tensor_copy(out=pj[:, ts(nh, 512)], in_=ps)
                    if nh == 1:
                        nc.sync.dma_start(out=rs_in[qc, ts(tt, 128), :], in_=pj)
                items.append(fn)

        def fn_rs():
            nc.gpsimd.collective_compute(
                "ReduceScatter",
                add_op,
                replica_groups=GROUPS,
                ins=[rs_in[qc].flatten()],
                outs=[rs_out[qc].flatten()],
            )
        items.append(fn_rs)
        return items

    def attention_unit(qc, pr, deferred):
        ctx_e = at_ctx.tile([65, 512], F32, tag="ctx_e")
        ctx_o = at_ctx.tile([65, 512], F32, tag="ctx_o")

        # software-pipelined: emit ctx(kt-1) after scores(kt) so the PE's
        # in-order stream never stalls on the exp of the current tile
        def emit_ctx(kt, ex):
            nc.tensor.matmul(
                ctx_e,
                lhsT=vaug_sb[:, kt, 2 * pr, :],
                rhs=ex[:, 0:512],
                start=(kt == 0), stop=(kt == 15),
            )
            nc.tensor.matmul(
                ctx_o,
                lhsT=vaug_sb[:, kt, 2 * pr + 1, :],
                rhs=ex[:, 512:1024],
                start=(kt == 0), stop=(kt == 15),
            )

        prev = None
        for kt in range(16):
            sc = at_sc.tile([128, 1024], F32, tag="sc")
            nc.tensor.matmul(
                sc[:, 0:512],
                lhsT=kT_sb[0:64, pr, ts(kt, 128)],
                rhs=qT_sb[0:64, pr, ts(qc, 512)],
            )
            nc.tensor.matmul(
                sc[:, 512:1024],
                lhsT=kT_sb[64:128, pr, ts(kt, 128)],
                rhs=qT_sb[64:128, pr, ts(qc, 512)],
            )
            ex = at_sb.tile([128, 1024], BF16, tag="ex")
            nc.scalar.activation(
                out=ex, in_=sc[:, :], func=Exp,
                bias=msk_sb[:, kt:kt + 1], scale=0.125,
            )
            if prev is not None:
                emit_ctx(*prev)
            if kt >= 4 and deferred:
                deferred.pop(0)()
            prev = (kt, ex)
        emit_ctx(*prev)
        return norm_unit(qc, pr, ctx_e, ctx_o)

    deferred = []
    for qc in range(4):
        units = []
        units.append(attention_unit(qc, 0, deferred))
        units.append(attention_unit(qc, 1, deferred))
        assert not deferred, f"{len(deferred)} deferred items left at qc={qc}"
        deferred = post_items(qc, units)
    # last chunk's post work has nothing left to hide behind
    for fn in deferred:
        fn()

    # ---- residual + bias + LayerNorm, off the attention/RS critical path.
    # rstd = exp(-0.5*ln(var+eps)) keeps all ScalarE work in the
    # natural_log_exp table set (no reloads between exp batches).
    for qc in range(4):
        x = fin_sb.tile([128, H], F32, tag="x")
        nc.sync.dma_start(out=x, in_=rs_out[qc])
        nc.vector.tensor_tensor(out=x, in0=x, in1=hso_sb[:, qc, :], op=add_op)
        nc.vector.tensor_tensor(out=x, in0=x, in1=bdb_sb, op=add_op)
        stats = fin_sb.tile([128, 2, 6], F32, tag="stats")
        for i in range(2):
            nc.vector.bn_stats(out=stats[:, i, :], in_=x[:, ts(i, 512)])
        mv = fin_sb.tile([128, 2], F32, tag="mv")
        nc.vector.bn_aggr(out=mv, in_=stats)
        lv = fin_sb.tile([128, 1], F32, tag="lv")
        nc.scalar.activation(out=lv, in_=mv[:, 1:2], func=Ln, bias=eps_sb, scale=1.0)
        rinv = fin_sb.tile([128, 1], F32, tag="rinv")
        nc.scalar.activation(out=rinv, in_=lv, func=Exp, scale=-0.5)
        nc.vector.tensor_scalar(
            out=x, in0=x, scalar1=mv[:, 0:1], scalar2=rinv,
            op0=sub_op, op1=mul_op,
        )
        nc.vector.tensor_tensor(out=x, in0=x, in1=gmb_sb, op=mul_op)
        nc.vector.tensor_tensor(out=x, in0=x, in1=btb_sb, op=add_op)
        nc.sync.dma_start(out=out_ap[qc], in_=x)

    fin_sb.release()
    at_sb.release()
    at_rbc.release()
    at_ctx.release()
    at_sc.release()
    dram.release()
    persist.release()


_NC_CACHE = {}


def _get_nc():
    if "nc" not in _NC_CACHE:
        _NC_CACHE["nc"] = build_nc()
    return _NC_CACHE["nc"]


def _bf16(x):
    import ml_dtypes
    return np.ascontiguousarray(x.astype(ml_dtypes.bfloat16))


def shard_inputs(inputs):
    import ml_dtypes
    hs = np.ascontiguousarray(np.asarray(inputs["hidden_states"], dtype=np.float32))
    mask = np.asarray(inputs["attention_mask"], dtype=np.float32)
    Wq = np.asarray(inputs["Wq"], dtype=np.float32)
    Wk = np.asarray(inputs["Wk"], dtype=np.float32)
    Wv = np.asarray(inputs["Wv"], dtype=np.float32)
    Wd = np.asarray(inputs["Wd"], dtype=np.float32)
    bq = np.asarray(inputs["bq"], dtype=np.float32)
    bk = np.asarray(inputs["bk"], dtype=np.float32)
    bv = np.asarray(inputs["bv"], dtype=np.float32)
    bd = np.ascontiguousarray(np.asarray(inputs["bd"], dtype=np.float32))
    gam = np.ascontiguousarray(np.asarray(inputs["ln_gamma"], dtype=np.float32))
    bet = np.ascontiguousarray(np.asarray(inputs["ln_beta"], dtype=np.float32))

    hsT = [_bf16(hs[b].T) for b in range(B)]
    mask_b = [np.ascontiguousarray(mask[b, 0, 0, :]) for b in range(B)]
    ones_fr = np.ones((128, 128), np.float32)
    ones_bf = np.ones((128, 64), ml_dtypes.bfloat16)

    in_maps = []
    for c in range(NCORES):
        b, g = c // 4, c % 4
        sl = slice(256 * g, 256 * g + 256)
        # owned token rows: 512*qc + 128*g .. +128 for qc in 0..3
        hs_own = np.stack(
            [hs[b, 512 * qc + 128 * g: 512 * qc + 128 * g + 128] for qc in range(4)]
        )
        in_maps.append({
            "hsT": hsT[b],
            "hs_own": np.ascontiguousarray(hs_own),
            "wqT": _bf16(Wq[sl, :].T),
            "wkT": _bf16(Wk[sl, :].T),
            "wvT": _bf16(Wv[sl, :].T),
            "wdT": _bf16(Wd[:, sl].T),
            "bq_s": np.ascontiguousarray(bq[sl]),
            "bk_s": np.ascontiguousarray(bk[sl]),
            "bv_s": np.ascontiguousarray(bv[sl]),
            "bd_f": bd,
            "gamma_f": gam,
            "beta_f": bet,
            "mask_b": mask_b[b],
            "ones_fr": ones_fr,
            "ones_bf": ones_bf,
        })
    return in_maps


def assemble(results):
    out = np.zeros((B, S, H), np.float32)
    for c in range(NCORES):
        b, g = c // 4, c % 4
        for qc in range(4):
            r0 = 512 * qc + 128 * g
            out[b, r0:r0 + 128, :] = results[c]["out_chunk"][qc]
    return out


LAST_RESULT = None


def kernel(**inputs):
    global LAST_RESULT
    from concourse.bass_utils import run_bass_kernel_spmd

    nc = _get_nc()
    in_maps = shard_inputs(inputs)
    trace = bool(int(os.environ.get("KERNEL_TRACE", "0")))
    res = run_bass_kernel_spmd(nc, in_maps, list(range(NCORES)), trace=trace)
    LAST_RESULT = res
    return assemble(res.results)


def simulate(inputs):
    """CoreSim-based check (no hardware)."""
    from concourse.bass_interp import MultiCoreSim

    nc = _get_nc()
    in_maps = shard_inputs(inputs)
    sim = MultiCoreSim(nc, NCORES)
    for c in range(NCORES):
        for k, v in in_maps[c].items():
            sim.cores[c].tensor(k)[:] = v
    sim.simulate(check_with_hw=False)
    results = [{"out_chunk": np.array(sim.cores[c].tensor("out_chunk"))}
               for c in range(NCORES)]
    return assemble(results)


# revision 25
# speedup vs baseline: 1.6824x; 1.0666x over previous
"""Trainium2 Bass kernel for ALBERT attention (B=2, S=2048, H=1024, NH=16).

Sharding over 8 NeuronCores: 2 batches x 4 head-groups (tensor parallel over
heads within each batch).  Core c handles batch b = c//4 and heads
[4g, 4g+4) where g = c%4.  The kernel pipelines over four 512-token chunks:
for each chunk it runs attention (both head pairs), the partial output
projection, and a ReduceScatter(add) over the batch's 4-core group that both
sums the head-group partials and scatters token ownership; the RS of chunk i
overlaps the attention of chunk i+1.  Core (b, g) ends up owning token rows
512*qc + 128*g .. +128 for qc in 0..3, applies residual + bias + LayerNorm,
and writes those four [128, 1024] slices.

Matmuls run in bf16 (inputs host-cast; fp32 PSUM accumulation).  The softmax
normalization (1/l) and the tiny K=1 broadcast matmuls stay float32r.

Per-core dataflow:
  hsT [1024, 2048] bf16   (host-pretransposed hidden states of its batch)
  qT/kT = W.T-slices @ hsT          (feature-major, head pairs stacked 64+64)
  v     = hsT.T @ WvT-slice         (token-major) + ones column per head
  per 512-token q chunk, per head pair, per 128-key tile:
      scoresT[key, q] = k @ qT   (two heads row-packed, K=64)
      expT = exp(0.125*scoresT + mask[key])   (ScalarE, mask as bias)
      ctx~T[65, q] += [v | 1].T @ expT        (PSUM-accumulated over keys)
  ctxT_h = ctx~T[:64] / ctx~T[64]   (merged reciprocal + K=1 bcast matmul)
  partial(qc) = ctxT(qc).T @ WdT-slice -> ReduceScatter(qc) over group of 4
  out = LN(rs + hs_own + bd) * gamma + beta
"""

import os
import sys

import numpy as np

for _p in ("/opt/trn_rl_repo",):
    if _p not in sys.path:
        sys.path.insert(0, _p)

import concourse.bass as bass
import concourse.mybir as mybir
import concourse.tile as tile
from concourse import bacc
from concourse.bass import ts

F32 = mybir.dt.float32
F32R = mybir.dt.float32r
BF16 = mybir.dt.bfloat16

H, NH, HD = 1024, 16, 64
B, S = 2, 2048
NCORES = 8
GROUPS = [[0, 1, 2, 3], [4, 5, 6, 7]]
CHUNK = 512          # tokens per pipelined chunk (and per-core output rows)
EPS = 1e-12


def build_nc():
    nc = bacc.Bacc(
        "TRN2",
        target_bir_lowering=False,
        debug=False,
        num_devices=NCORES,
    )

    hsT_d = nc.dram_tensor("hsT", [H, S], BF16, kind="ExternalInput")
    hso_d = nc.dram_tensor("hs_own", [4, 128, H], F32, kind="ExternalInput")
    wq_d = nc.dram_tensor("wqT", [H, 256], BF16, kind="ExternalInput")
    wk_d = nc.dram_tensor("wkT", [H, 256], BF16, kind="ExternalInput")
    wv_d = nc.dram_tensor("wvT", [H, 256], BF16, kind="ExternalInput")
    wd_d = nc.dram_tensor("wdT", [256, H], BF16, kind="ExternalInput")
    bq_d = nc.dram_tensor("bq_s", [256], F32, kind="ExternalInput")
    bk_d = nc.dram_tensor("bk_s", [256], F32, kind="ExternalInput")
    bv_d = nc.dram_tensor("bv_s", [256], F32R, kind="ExternalInput")
    bd_d = nc.dram_tensor("bd_f", [H], F32R, kind="ExternalInput")
    gam_d = nc.dram_tensor("gamma_f", [H], F32R, kind="ExternalInput")
    bet_d = nc.dram_tensor("beta_f", [H], F32R, kind="ExternalInput")
    msk_d = nc.dram_tensor("mask_b", [S], F32, kind="ExternalInput")
    onr_d = nc.dram_tensor("ones_fr", [128, 128], F32R, kind="ExternalInput")
    onb_d = nc.dram_tensor("ones_bf", [128, 64], BF16, kind="ExternalInput")
    out_d = nc.dram_tensor("out_chunk", [4, 128, H], F32, kind="ExternalOutput")

    with tile.TileContext(nc) as tc:
        _body(tc, hsT_d, hso_d, wq_d, wk_d, wv_d, wd_d, bq_d, bk_d, bv_d,
              bd_d, gam_d, bet_d, msk_d, onr_d, onb_d, out_d)
    nc.compile()
    return nc


def _body(tc, hsT_d, hso_d, wq_d, wk_d, wv_d, wd_d, bq_d, bk_d, bv_d,
          bd_d, gam_d, bet_d, msk_d, onr_d, onb_d, out_d):
    nc = tc.nc
    Exp = mybir.ActivationFunctionType.Exp
    Ln = mybir.ActivationFunctionType.Ln
    add_op = mybir.AluOpType.add
    sub_op = mybir.AluOpType.subtract
    mul_op = mybir.AluOpType.mult

    # ---------------- persistent SBUF ----------------
    persist = tc.alloc_tile_pool(name="persist", bufs=1)
    qT_sb = persist.tile([128, 2, S], BF16)      # [dim-in-pair, pair, tok]
    kT_sb = persist.tile([128, 2, S], BF16)
    vaug_sb = persist.tile([128, 16, 4, 65], BF16)  # [key-in-tile, keytile, head, 64v+1]
    ctxT_sb = persist.tile([128, 2, S], BF16)    # normalized ctx, feature-major
    cxo_sb = persist.tile([64, 2, S], BF16)      # odd heads before partition shift
    wd_sb = persist.tile([128, 2, H], BF16)
    bq_sb = persist.tile([128, 2], F32)
    bk_sb = persist.tile([128, 2], F32)
    msk_sb = persist.tile([128, 16], F32)
    onr_sb = persist.tile([128, 128], F32R)
    bv_sb = persist.tile([1, 256], F32R)
    bd_sb = persist.tile([1, H], F32R)
    gam_sb = persist.tile([1, H], F32R)
    bet_sb = persist.tile([1, H], F32R)
    eps_sb = persist.tile([128, 1], F32)

    nc.vector.memset(eps_sb, EPS)
    # ---------------- load pool (released after QKV) ----------------
    load = tc.alloc_tile_pool(name="load", bufs=1)
    hsT_sb = load.tile([128, 8, S], BF16)        # [feat-in-chunk, featchunk, tok]
    wq_sb = load.tile([128, 8, 256], BF16)
    wk_sb = load.tile([128, 8, 256], BF16)
    wv_sb = load.tile([128, 8, 256], BF16)

    # issue order favors the first QKV tiles: wq/wk + first token chunk first
    hsT_src = hsT_d.ap().rearrange("(c p) (q w) -> q p c w", p=128, w=512)
    nc.sync.dma_start(out=wq_sb, in_=wq_d.ap().rearrange("(c p) d -> p c d", p=128))
    nc.sync.dma_start(out=wk_sb, in_=wk_d.ap().rearrange("(c p) d -> p c d", p=128))
    nc.sync.dma_start(out=hsT_sb[:, :, ts(0, 512)], in_=hsT_src[0])
    nc.sync.dma_start(out=wv_sb, in_=wv_d.ap().rearrange("(c p) d -> p c d", p=128))
    nc.sync.dma_start(out=bq_sb, in_=bq_d.ap().rearrange("(c p) -> p c", p=128))
    nc.sync.dma_start(out=bk_sb, in_=bk_d.ap().rearrange("(c p) -> p c", p=128))
    nc.sync.dma_start(out=bv_sb, in_=bv_d.ap().unsqueeze(0))
    nc.sync.dma_start(out=onr_sb, in_=onr_d.ap())
    for t4 in range(1, 4):
        nc.sync.dma_start(out=hsT_sb[:, :, ts(t4, 512)], in_=hsT_src[t4])
    nc.sync.dma_start(
        out=vaug_sb[:, :, :, 64:65],
        in_=onb_d.ap().rearrange("p (a b) -> p a b", a=16).unsqueeze(3),
    )
    nc.sync.dma_start(out=msk_sb, in_=msk_d.ap().rearrange("(t p) -> p t", p=128))
    nc.sync.dma_start(out=wd_sb, in_=wd_d.ap().rearrange("(c p) d -> p c d", p=128))
    nc.sync.dma_start(out=bd_sb, in_=bd_d.ap().unsqueeze(0))
    nc.sync.dma_start(out=gam_sb, in_=gam_d.ap().unsqueeze(0))
    nc.sync.dma_start(out=bet_sb, in_=bet_d.ap().unsqueeze(0))

    # ---------------- QKV projections ----------------
    qkv_ps = tc.alloc_tile_pool(name="qkv_ps", bufs=3, space="PSUM")

    # kc-outer with 4 live PSUM banks: one LDWEIGHTS per (kc, pr) serves 4
    # matmuls, and consecutive matmuls hit different banks so drains overlap
    for pr in range(2):
        for w_sb, b_sb, o_sb in (
            (wq_sb, bq_sb, qT_sb),
            (wk_sb, bk_sb, kT_sb),
        ):
            pss = [qkv_ps.tile([128, 512], F32, tag="qk_ps", bufs=4,
                                name=f"qk_ps_{i}")
                   for i in range(4)]
            for kc in range(8):
                for t4 in range(4):
                    nc.tensor.matmul(
                        pss[t4],
                        lhsT=w_sb[:, kc, ts(pr, 128)],
                        rhs=hsT_sb[:, kc, ts(t4, 512)],
                        start=(kc == 0),
                        stop=(kc == 7),
                    )
            for t4 in range(4):
                nc.vector.tensor_scalar_add(
                    out=o_sb[:, pr, ts(t4, 512)], in0=pss[t4],
                    scalar1=b_sb[:, pr:pr + 1],
                )
    for t4 in range(4):
        for tp in range(2):          # pairs of 128-token tiles, interleaved
            ta, tb = 4 * t4 + 2 * tp, 4 * t4 + 2 * tp + 1
            psa = qkv_ps.tile([128, 256], F32, tag="v_ps")
            psb = qkv_ps.tile([128, 256], F32, tag="v_ps")
            for kc in range(8):
                nc.tensor.matmul(
                    psa,
                    lhsT=hsT_sb[:, kc, ts(ta, 128)],
                    rhs=wv_sb[:, kc, :],
                    start=(kc == 0),
                    stop=False,
                )
                nc.tensor.matmul(
                    psb,
                    lhsT=hsT_sb[:, kc, ts(tb, 128)],
                    rhs=wv_sb[:, kc, :],
                    start=(kc == 0),
                    stop=False,
                )
            for t16, ps in ((ta, psa), (tb, psb)):
                nc.tensor.matmul(    # + bv broadcast over tokens (K=1, f32r)
                    ps,
                    lhsT=onr_sb[0:1, 0:128],
                    rhs=bv_sb[0:1, :],
                    start=False,
                    stop=True,
                )
                nc.vector.tensor_copy(
                    out=vaug_sb[:, t16, :, 0:64],
                    in_=ps.rearrange("p (h d) -> p h d", h=4),
                )

    qkv_ps.release()
    load.release()

    # ------- attention -> dense -> chunked ReduceScatter -> LayerNorm -------
    dram = tc.alloc_tile_pool(name="dram", bufs=1, space="DRAM")
    rs_in = dram.tile([4, CHUNK, H], F32)        # per-qc partial projections
    rs_out = dram.tile([4, 128, H], F32)         # per-qc owned token rows

    at_sc = tc.alloc_tile_pool(name="at_sc", bufs=2, space="PSUM")    # 4 banks
    at_ctx = tc.alloc_tile_pool(name="at_ctx", bufs=1, space="PSUM")  # 2 banks
    at_rbc = tc.alloc_tile_pool(name="at_rbc", bufs=2, space="PSUM")  # 2 banks
    at_sb = tc.alloc_tile_pool(name="at_sb", bufs=3)
    fin_sb = tc.alloc_tile_pool(name="fin_sb", bufs=2)

    hso_sb = fin_sb.tile([128, 4, H], F32, bufs=1)
    nc.sync.dma_start(out=hso_sb, in_=hso_d.ap().rearrange("q p d -> p q d"))

    # broadcast bd / gamma / beta across partitions via K=1 matmuls (f32r)
    bdb_sb = fin_sb.tile([128, H], F32, bufs=1)
    gmb_sb = fin_sb.tile([128, H], F32, bufs=1)
    btb_sb = fin_sb.tile([128, H], F32, bufs=1)
    for src, dst in ((bd_sb, bdb_sb), (gam_sb, gmb_sb), (bet_sb, btb_sb)):
        for nh in range(2):
            pb = at_rbc.tile([128, 512], F32, tag="rbc")
            nc.tensor.matmul(
                pb,
                lhsT=onr_sb[0:1, 0:128],
                rhs=src[0:1, ts(nh, 512)],
            )
            nc.vector.tensor_copy(out=dst[:, ts(nh, 512)], in_=pb)

    out_ap = out_d.ap()

    def norm_unit(qc, pr, ctx_e, ctx_o):
        """DVE-only epilogue of an attention unit: copy ctx~ out of PSUM
        (freeing the banks for the next unit) and start the 1/l chain.
        Returns (cse, cso, lrecr) plus closures of deferred PE work."""
        cse = at_sb.tile([65, 512], F32, tag="cse", bufs=2)
        cso = at_sb.tile([65, 512], F32, tag="cso", bufs=2)
        nc.vector.tensor_copy(out=cse, in_=ctx_e)
        nc.vector.tensor_copy(out=cso, in_=ctx_o)
        lrec = at_sb.tile([128, 1024], F32, tag="lrec", bufs=2)
        nc.vector.tensor_copy(out=lrec[64:65, 0:512], in_=cse[64:65, :])
        nc.vector.tensor_copy(out=lrec[64:65, 512:1024], in_=cso[64:65, :])
        # 1/l = exp(-ln(l)) on ScalarE: keeps the slow single-lane reciprocal
        # out of the DVE FIFO (deferred muls queue behind it otherwise), and
        # Ln/Exp share one ACT table set with the attention exps.
        lnl = at_sb.tile([128, 1024], F32, tag="lnl", bufs=2)
        nc.scalar.activation(out=lnl[64:65, :], in_=lrec[64:65, :], func=Ln)
        lrecr = at_sb.tile([128, 1024], F32R, tag="lrecr", bufs=2)
        nc.scalar.activation(out=lrecr[64:65, :], in_=lnl[64:65, :], func=Exp,
                             scale=-1.0)
        return cse, cso, lrecr

    def post_items(qc, units):
        """Deferred PE work for chunk qc: softmax normalization broadcasts,
        the partition shift, the partial dense, and the ReduceScatter.
        Returned as small closures to interleave into the next unit's
        key-tile loop (the PE has slack under the ACT-bound exp stream)."""
        items = []
        for pr in range(2):
            cse, cso, lrecr = units[pr]

            def fn(pr=pr, cse=cse, cso=cso, lrecr=lrecr):
                for hodd, csrc in ((0, cse), (1, cso)):
                    rbc = at_rbc.tile([128, 512], F32, tag="rbc")
                    nc.tensor.matmul(
                        rbc[0:64, :],
                        lhsT=onr_sb[64:65, 0:64],
                        rhs=lrecr[64:65, ts(hodd, 512)],
                        tile_position=(64, 0),
                    )
                    dst = (ctxT_sb if hodd == 0 else cxo_sb)[0:64, pr, ts(qc, 512)]
                    nc.vector.tensor_tensor(
                        out=dst, in0=csrc[0:64, :], in1=rbc[0:64, :], op=mul_op,
                    )
                # odd-head rows into ctxT partitions 64..127 (only DMA can
                # cross partitions)
                nc.sync.dma_start(
                    out=ctxT_sb[64:128, pr, ts(qc, 512)],
                    in_=cxo_sb[0:64, pr, ts(qc, 512)],
                )
            items.append(fn)
        for tt in range(4):
            pj = fin_sb.tile([128, H], F32, tag="pj", bufs=3)
            for nh in range(2):
                def fn(tt=tt, nh=nh, pj=pj):
                    ps = at_rbc.tile([128, 512], F32, tag="rbc")
                    for pr in range(2):
                        nc.tensor.matmul(
                            ps,
                            lhsT=ctxT_sb[:, pr, ts(4 * qc + tt, 128)],
                            rhs=wd_sb[:, pr, ts(nh, 512)],
                            start=(pr == 0),
                            stop=(pr == 1),
                        )
                    nc.vector.tensor_copy(out=pj[:, ts(nh, 512)], in_=ps)
                    if nh == 1:
                        nc.sync.dma_start(out=rs_in[qc, ts(tt, 128), :], in_=pj)
                items.append(fn)

        def fn_rs():
            nc.gpsimd.collective_compute(
                "ReduceScatter",
                add_op,
                replica_groups=GROUPS,
                ins=[rs_in[qc].flatten()],
                outs=[rs_out[qc].flatten()],
            )
        items.append(fn_rs)
        return items

    def attention_unit(qc, pr, deferred):
        ctx_e = at_ctx.tile([65, 512], F32, tag="ctx_e")
        ctx_o = at_ctx.tile([65, 512], F32, tag="ctx_o")

        # software-pipelined: emit ctx(kt-1) after scores(kt) so the PE's
        # in-order stream never stalls on the exp of the current tile
        def emit_ctx(kt, ex):
            nc.tensor.matmul(
                ctx_e,
                lhsT=vaug_sb[:, kt, 2 * pr, :],
                rhs=ex[:, 0:512],
                start=(kt == 0), stop=(kt == 15),
            )
            nc.tensor.matmul(
                ctx_o,
                lhsT=vaug_sb[:, kt, 2 * pr + 1, :],
                rhs=ex[:, 512:1024],
                start=(kt == 0), stop=(kt == 15),
            )

        prev = None
        for kt in range(16):
            sc = at_sc.tile([128, 1024], F32, tag="sc")
            nc.tensor.matmul(
                sc[:, 0:512],
                lhsT=kT_sb[0:64, pr, ts(kt, 128)],
                rhs=qT_sb[0:64, pr, ts(qc, 512)],
            )
            nc.tensor.matmul(
                sc[:, 512:1024],
                lhsT=kT_sb[64:128, pr, ts(kt, 128)],
                rhs=qT_sb[64:128, pr, ts(qc, 512)],
            )
            ex = at_sb.tile([128, 1024], BF16, tag="ex")
            nc.scalar.activation(
                out=ex, in_=sc[:, :], func=Exp,
                bias=msk_sb[:, kt:kt + 1], scale=0.125,
            )
            if prev is not None:
                emit_ctx(*prev)
            if kt >= 4 and deferred:
                deferred.pop(0)()
            prev = (kt, ex)
        emit_ctx(*prev)
        return norm_unit(qc, pr, ctx_e, ctx_o)

    deferred = []
    for qc in range(4):
        units = []
        units.append(attention_unit(qc, 0, deferred))
        units.append(attention_unit(qc, 1, deferred))
        assert not deferred, f"{len(deferred)} deferred items left at qc={qc}"
        deferred = post_items(qc, units)
    # last chunk's post work has nothing left to hide behind
    for fn in deferred:
        fn()

    # ---- residual + bias + LayerNorm, off the attention/RS critical path.
    # rstd = exp(-0.5*ln(var+eps)) keeps all ScalarE work in the
    # natural_log_exp table set (no reloads between exp batches).
    for qc in range(4):
        x = fin_sb.tile([128, H], F32, tag="x")
        nc.sync.dma_start(out=x, in_=rs_out[qc])
        nc.vector.tensor_tensor(out=x, in0=x, in1=hso_sb[:, qc, :], op=add_op)
        nc.vector.tensor_tensor(out=x, in0=x, in1=bdb_sb, op=add_op)
        stats = fin_sb.tile([128, 2, 6], F32, tag="stats")
        for i in range(2):
            nc.vector.bn_stats(out=stats[:, i, :], in_=x[:, ts(i, 512)])
        mv = fin_sb.tile([128, 2], F32, tag="mv")
        nc.vector.bn_aggr(out=mv, in_=stats)
        lv = fin_sb.tile([128, 1], F32, tag="lv")
        nc.scalar.activation(out=lv, in_=mv[:, 1:2], func=Ln, bias=eps_sb, scale=1.0)
        rinv = fin_sb.tile([128, 1], F32, tag="rinv")
        nc.scalar.activation(out=rinv, in_=lv, func=Exp, scale=-0.5)
        nc.vector.tensor_scalar(
            out=x, in0=x, scalar1=mv[:, 0:1], scalar2=rinv,
            op0=sub_op, op1=mul_op,
        )
        nc.vector.tensor_tensor(out=x, in0=x, in1=gmb_sb, op=mul_op)
        nc.vector.tensor_tensor(out=x, in0=x, in1=btb_sb, op=add_op)
        nc.sync.dma_start(out=out_ap[qc], in_=x)

    fin_sb.release()
    at_sb.release()
    at_rbc.release()
    at_ctx.release()
    at_sc.release()
    dram.release()
    persist.release()


_NC_CACHE = {}


def _get_nc():
    if "nc" not in _NC_CACHE:
        _NC_CACHE["nc"] = build_nc()
    return _NC_CACHE["nc"]


def _bf16(x):
    import ml_dtypes
    return np.ascontiguousarray(x.astype(ml_dtypes.bfloat16))


def shard_inputs(inputs):
    import ml_dtypes
    hs = np.ascontiguousarray(np.asarray(inputs["hidden_states"], dtype=np.float32))
    mask = np.asarray(inputs["attention_mask"], dtype=np.float32)
    Wq = np.asarray(inputs["Wq"], dtype=np.float32)
    Wk = np.asarray(inputs["Wk"], dtype=np.float32)
    Wv = np.asarray(inputs["Wv"], dtype=np.float32)
    Wd = np.asarray(inputs["Wd"], dtype=np.float32)
    bq = np.asarray(inputs["bq"], dtype=np.float32)
    bk = np.asarray(inputs["bk"], dtype=np.float32)
    bv = np.asarray(inputs["bv"], dtype=np.float32)
    bd = np.ascontiguousarray(np.asarray(inputs["bd"], dtype=np.float32))
    gam = np.ascontiguousarray(np.asarray(inputs["ln_gamma"], dtype=np.float32))
    bet = np.ascontiguousarray(np.asarray(inputs["ln_beta"], dtype=np.float32))

    hsT = [_bf16(hs[b].T) for b in range(B)]
    mask_b = [np.ascontiguousarray(mask[b, 0, 0, :]) for b in range(B)]
    ones_fr = np.ones((128, 128), np.float32)
    ones_bf = np.ones((128, 64), ml_dtypes.bfloat16)

    in_maps = []
    for c in range(NCORES):
        b, g = c // 4, c % 4
        sl = slice(256 * g, 256 * g + 256)
        # owned token rows: 512*qc + 128*g .. +128 for qc in 0..3
        hs_own = np.stack(
            [hs[b, 512 * qc + 128 * g: 512 * qc + 128 * g + 128] for qc in range(4)]
        )
        in_maps.append({
            "hsT": hsT[b],
            "hs_own": np.ascontiguousarray(hs_own),
            "wqT": _bf16(Wq[sl, :].T),
            "wkT": _bf16(Wk[sl, :].T),
            "wvT": _bf16(Wv[sl, :].T),
            "wdT": _bf16(Wd[:, sl].T),
            "bq_s": np.ascontiguousarray(bq[sl]),
            "bk_s": np.ascontiguousarray(bk[sl]),
            "bv_s": np.ascontiguousarray(bv[sl]),
            "bd_f": bd,
            "gamma_f": gam,
            "beta_f": bet,
            "mask_b": mask_b[b],
            "ones_fr": ones_fr,
            "ones_bf": ones_bf,
        })
    return in_maps


def assemble(results):
    out = np.zeros((B, S, H), np.float32)
    for c in range(NCORES):
        b, g = c // 4, c % 4
        for qc in range(4):
            r0 = 512 * qc + 128 * g
            out[b, r0:r0 + 128, :] = results[c]["out_chunk"][qc]
    return out


LAST_RESULT = None


def kernel(**inputs):
    global LAST_RESULT
    from concourse.bass_utils import run_bass_kernel_spmd

    nc = _get_nc()
    in_maps = shard_inputs(inputs)
    trace = bool(int(os.environ.get("KERNEL_TRACE", "0")))
    res = run_bass_kernel_spmd(nc, in_maps, list(range(NCORES)), trace=trace)
    LAST_RESULT = res
    return assemble(res.results)


def simulate(inputs):
    """CoreSim-based check (no hardware)."""
    from concourse.bass_interp import MultiCoreSim

    nc = _get_nc()
    in_maps = shard_inputs(inputs)
    sim = MultiCoreSim(nc, NCORES)
    for c in range(NCORES):
        for k, v in in_maps[c].items():
            sim.cores[c].tensor(k)[:] = v
    sim.simulate(check_with_hw=False)
    results = [{"out_chunk": np.array(sim.cores[c].tensor("out_chunk"))}
               for c in range(NCORES)]
    return assemble(results)
